# revision 10
# baseline (speedup 1.0000x reference)
"""Trainium2 Bass kernel for nn_CompILE (CompILE forward), 8-core data parallel.

Sharding: batch B=256 split across 8 NeuronCores (32 each); weights replicated.
Per core:
  P0: transpose embed_w on-device; build gate-embedding table
      EW[v, :] = w_ih_r @ embed_w[v] + (b_ih + b_hh) (gate order i,f,o,g) in DRAM.
  LSTM (4 segments x 128 sequential steps): gates = h @ w_hh_r.T + EW[idx_t]
      (PE matmuls, fp32r moving operand; EW row fetched by indirect DMA and
      added via identity matmul); sigmoid/tanh on ACT; cell update on DVE;
      PE-transposed h kept in SBUF as encT [H, b*T+t] for the boundary heads.
  Boundary: hid = relu(wb1 @ encT + b); lb = wb2 . hid; gumbel softmax as
      exp(lb - C)*exp(gumbel) normalized per row (host ships exp(gumbel),
      threefry seed 42, matching the reference); cumsum via tensor_tensor_scan;
      mask = exp(accumulated log cumsum); readout/z/decode in transposed form.
Host side only reorders/shards/transposes parameter layouts, precomputes the
fixed-seed noise constants, and reassembles device outputs.
"""
import os
import sys
import numpy as np

if "/opt/trn_rl_repo" not in sys.path:
    sys.path.insert(0, "/opt/trn_rl_repo")

EPS = 1e-17
NEG_INF = -1e30
B, T, V, H, L, S = 256, 128, 512, 256, 64, 4
BS = 32
BT = BS * T  # per-core rows, b-major: col = b*T + t
CSOFT = 10.0
NCORES = 8

_CACHE = {}


def _split_sync_waits(nc, limit=1):
    """walrus codegen in this build rejects >1 sync wait per instruction;
    hoist excess waits onto same-engine no-ops inserted just before."""
    import bass_rust
    import concourse.mybir as mybir
    n = 0
    for fn in nc.m.functions:
        for bb in fn.blocks:
            out = []
            changed = False
            for inst in bb.instructions:
                si = getattr(inst, "sync_info", None)
                ow = list(si.on_wait) if (si is not None and si.on_wait) else []
                if len(ow) > limit:
                    keep = ow[-limit:]
                    extra = ow[:-limit]
                    for j in range(0, len(extra), limit):
                        nop = bass_rust.InstNoOp(
                            name=f"I-wsplit-{n}", ins=[], outs=[])
                        n += 1
                        nop.engine = inst.engine
                        nop.sync_info = mybir.SyncInfo(
                            on_wait=extra[j:j + limit], on_update=[])
                        out.append(nop)
                    si.on_wait = keep
                    changed = True
                out.append(inst)
            if changed:
                try:
                    bb.instructions = out
                except Exception:
                    bb.instructions.clear()
                    for x in out:
                        bb.instructions.append(x)
    return n


def build_bass():
    import concourse.bass as bass
    import concourse.mybir as mybir
    import concourse.tile as tile
    from concourse.bass import IndirectOffsetOnAxis

    f32 = mybir.dt.float32
    f32r = mybir.dt.float32r
    i32 = mybir.dt.int32
    AF = mybir.ActivationFunctionType
    OP = mybir.AluOpType
    AX = mybir.AxisListType

    nc = bass.Bass("TRN2", target_bir_lowering=False, debug=False,
                   num_devices=NCORES)

    def din(name, shape, dtype=f32):
        return nc.dram_tensor(name, shape, dtype, kind="ExternalInput")

    def dout(name, shape, dtype=f32):
        return nc.dram_tensor(name, shape, dtype, kind="ExternalOutput")

    embw = din("embw", [V, H])
    wihT = din("wihT", [2, 128, 4 * H], f32r)
    bihs = din("bihs", [1, 4 * H])
    bhhs = din("bhhs", [1, 4 * H])
    whhTd = din("whhTd", [2, 128, 4 * H], f32r)
    wb1Td = din("wb1Td", [2, 128, H], f32r)
    wb2d = din("wb2d", [128, 2], f32r)
    bb1d = din("bb1d", [128, 2])
    bb2d = din("bb2d", [1, 1])
    wz1Td = din("wz1Td", [2, 128, H])
    bz1d = din("bz1d", [128, 2])
    wz2Td = din("wz2Td", [2, 128, 2 * L])   # rows reordered to [log_var; mu]
    bz2d = din("bz2d", [128, 1])
    wd1Td = din("wd1Td", [L, H])
    bd1d = din("bd1d", [128, 2])
    wd2Td = din("wd2Td", [2, 128, V])
    bd2d = din("bd2d", [128, 4])
    idxd = din("idxd", [BS, T], i32)
    egd = din("egd", [3, BS, T])
    epzd = din("epzd", [L, 4 * BS])
    lend = din("lend", [BS, 1])
    iotad = din("iotad", [BS, T])
    id32d = din("id32d", [BS, BS])
    id32rd = din("id32rd", [BS, BS], f32r)
    id128d = din("id128d", [128, 128])

    ew_d = nc.dram_tensor("ew_d", [V, 4 * H], f32r)

    enc_o = dout("enc_o", [S, BS, T, H])
    rec_o = dout("rec_o", [S, BS, T, V])
    mask_o = dout("mask_o", [S - 1, BT])
    blog_o = dout("blog_o", [S - 1, BT])
    bsamp_o = dout("bsamp_o", [S, BT])
    zlog_o = dout("zlog_o", [S, 2 * L, BS])
    zsamp_o = dout("zsamp_o", [S, L, BS])

    def r(ap):
        return ap.bitcast(f32r)

    with tile.TileContext(nc) as tc:
      with tc.tile_pool(name="persist", bufs=1) as pp:
        def ptile(tag, shape, dtype=f32):
            return pp.tile(shape, dtype, tag=tag, name=tag)

        whh0 = ptile("whh0", [128, 4 * H], f32r); whh1 = ptile("whh1", [128, 4 * H], f32r)
        encT0 = ptile("encT0", [128, BT], f32r); encT1 = ptile("encT1", [128, BT], f32r)
        SG = ptile("SG", [BS, 1024])    # sig_ifo 0:768 | tanh_g 768:1024
        CW = ptile("CW", [BS, 512])     # cu 0:256 | tanh(cu) 256:512
        UV = ptile("UV", [BS, 512])
        Mrow = ptile("Mrow", [BS, T])
        idx_s = ptile("idx_s", [BS, T], i32)
        id32 = ptile("id32", [BS, BS])
        id32r = ptile("id32r", [BS, BS], f32r)
        id128 = ptile("id128", [128, 128])
        eps_s = ptile("eps_s", [L, 4 * BS])
        wb1_0 = ptile("wb1_0", [128, H], f32r); wb1_1 = ptile("wb1_1", [128, H], f32r)
        wb2_s = ptile("wb2_s", [128, 2], f32r); bb1_s = ptile("bb1_s", [128, 2])
        bb2_s = ptile("bb2_s", [1, 1]); bb2c_s = ptile("bb2c_s", [1, 1])
        wz1_0 = ptile("wz1_0", [128, H]); wz1_1 = ptile("wz1_1", [128, H])
        bz1_s = ptile("bz1_s", [128, 2])
        wz2_0 = ptile("wz2_0", [128, 2 * L]); wz2_1 = ptile("wz2_1", [128, 2 * L])
        bz2_s = ptile("bz2_s", [128, 1])
        wd1_s = ptile("wd1_s", [L, H]); bd1_s = ptile("bd1_s", [128, 2])
        wd2_0 = ptile("wd2_0", [128, V]); wd2_1 = ptile("wd2_1", [128, V])
        bd2_s = ptile("bd2_s", [128, 4])
        LCA32 = ptile("LCA32", [BS, T]); LG32 = ptile("LG32", [BS, T])
        EG32 = ptile("EG32", [BS, T]); ones32 = ptile("ones32", [BS, T])
        iota32 = ptile("iota32", [BS, T]); len32c = ptile("len32c", [BS, 1])
        ones1 = ptile("ones1", [1, 128])
        epsb = ptile("epsb", [128, 1])
        EX32 = ptile("EX32", [BS, T]); SB32 = ptile("SB32", [BS, T])
        ssum = ptile("ssum", [BS, 1]); rsum = ptile("rsum", [BS, 1])
        PRD = ptile("PRD", [128, 512])
        rdT = ptile("rdT", [128, 2 * BS])
        ZT = ptile("ZT", [128, BS])
        ZW = ptile("ZW", [L, 3 * BS])
        szT_s = ptile("szT_s", [L, BS])
        z1_s = ptile("z1_s", [128, 2 * BS])
        d1_s = ptile("d1_s", [128, 2 * BS])
        pred_s = ptile("pred_s", [128, 4 * BS])
        prow = ptile("prow", [BS, V])

        sdma = nc.sync.dma_start
        for dst, src in [
            (whh0[:], whhTd[0]), (whh1[:], whhTd[1]),
            (idx_s[:], idxd[:, :]), (id32[:], id32d[:, :]), (id32r[:], id32rd[:, :]),
            (id128[:], id128d[:, :]), (eps_s[:], epzd[:, :]),
            (wb1_0[:], wb1Td[0]), (wb1_1[:], wb1Td[1]),
            (wb2_s[:], wb2d[:, :]), (bb1_s[:], bb1d[:, :]),
            (bb2_s[:], bb2d[:, :]),
            (wz1_0[:], wz1Td[0]), (wz1_1[:], wz1Td[1]), (bz1_s[:], bz1d[:, :]),
            (wz2_0[:], wz2Td[0]), (wz2_1[:], wz2Td[1]), (bz2_s[:], bz2d[:, :]),
            (wd1_s[:], wd1Td[:, :]), (bd1_s[:], bd1d[:, :]),
            (wd2_0[:], wd2Td[0]), (wd2_1[:], wd2Td[1]), (bd2_s[:], bd2d[:, :]),
            (iota32[:], iotad[:, :]), (len32c[:], lend[:, :]),
        ]:
            sdma(dst, src)
        nc.vector.memset(ones1[:], 1.0)
        nc.vector.memset(epsb[:], EPS)
        nc.vector.memset(ones32[:], 1.0)
        nc.vector.tensor_scalar_add(bb2c_s[:], bb2_s[:], -CSOFT)

        # ---------------- P0: EW table ----------------
        ew_dmas = []
        with (tc.tile_pool(name="p0sb", bufs=2) as p0sb,
              tc.tile_pool(name="p0ps", bufs=2, space="PSUM") as p0ps,
              tc.tile_pool(name="p0ps2", bufs=2, space="PSUM") as p0ps2):
            embT = [p0sb.tile([128, V], f32r, tag="embT0", name="embT0"),
                    p0sb.tile([128, V], f32r, tag="embT1", name="embT1")]
            wihs = [p0sb.tile([128, 4 * H], f32r, tag="wihs0", name="wihs0"),
                    p0sb.tile([128, 4 * H], f32r, tag="wihs1", name="wihs1")]
            sdma(wihs[0][:], wihT[0]); sdma(wihs[1][:], wihT[1])
            bsum = p0sb.tile([1, 4 * H], f32, tag="bsum", name="bsum")
            bih_t = p0sb.tile([1, 4 * H], f32, tag="bih_t", name="bih_t")
            bhh_t = p0sb.tile([1, 4 * H], f32, tag="bhh_t", name="bhh_t")
            sdma(bih_t[:], bihs[:, :]); sdma(bhh_t[:], bhhs[:, :])
            nc.vector.tensor_tensor(bsum[:], bih_t[:], bhh_t[:], op=OP.add)

            for vc in range(4):
                et = p0sb.tile([128, H], f32, tag="et", name="et")
                sdma(et[:], embw[128 * vc:128 * (vc + 1), :])
                for hc in range(2):
                    pt0 = p0ps.tile([128, 128], f32, tag="p0t", name="p0t")
                    nc.tensor.transpose(pt0[:], et[:, 128 * hc:128 * (hc + 1)],
                                        id128[:])
                    nc.scalar.copy(embT[hc][:, 128 * vc:128 * (vc + 1)], pt0[:])

            for vc in range(4):
                ewst = p0sb.tile([128, 4 * H], f32r, tag="ewst", name="ewst")
                for nb in range(2):
                    pe = p0ps2.tile([128, 512], f32, tag="p0e", name="p0e")
                    for k in range(2):
                        nc.tensor.matmul(
                            pe[:], lhsT=embT[k][:, 128 * vc:128 * (vc + 1)],
                            rhs=wihs[k][:, 512 * nb:512 * (nb + 1)],
                            start=(k == 0), stop=False)
                    nc.tensor.matmul(pe[:], lhsT=ones1[:, 0:128],
                                     rhs=bsum[:, 512 * nb:512 * (nb + 1)],
                                     start=False, stop=True)
                    nc.scalar.copy(ewst[:, 512 * nb:512 * (nb + 1)], pe[:])
                ew_dmas.append(sdma(ew_d[128 * vc:128 * (vc + 1), :], ewst[:]))

        # ---------------- segments ----------------
        with (tc.tile_pool(name="gx", bufs=6) as gxp,
              tc.tile_pool(name="h2", bufs=3) as h2p,
              tc.tile_pool(name="rows", bufs=4) as rows,
              tc.tile_pool(name="hidp", bufs=3) as hidp):

            for s in range(S):
                if s == 0:
                    nc.vector.memset(Mrow[:], 1.0)
                    nc.vector.memset(LCA32[:], 0.0)
                nc.vector.memset(CW[:, 0:256], 0.0)

                with (tc.tile_pool(name=f"pg{s}", bufs=2, space="PSUM") as pgp,
                      tc.tile_pool(name=f"pt{s}", bufs=2, space="PSUM") as ptp):
                    for t in range(T):
                        gx = gxp.tile([BS, 4 * H], f32r, tag="gx", name="gx")
                        gi = nc.gpsimd.indirect_dma_start(
                            out=gx[:], out_offset=None,
                            in_=ew_d[:, :],
                            in_offset=IndirectOffsetOnAxis(
                                ap=idx_s[:, t:t + 1], axis=0))
                        if s == 0 and t == 0:
                            for d in ew_dmas:
                                tile.add_dep_helper(gi.ins, d.ins,
                                                    reason="gather after EW")
                        pg = pgp.tile([BS, 4 * H], f32, tag="pg", name="pg")
                        for nb in range(2):
                            pgn = pg[:, 512 * nb:512 * (nb + 1)]
                            if t > 0:
                                nc.tensor.matmul(
                                    pgn, lhsT=encT0[:, t - 1:BT:T],
                                    rhs=whh0[:, 512 * nb:512 * (nb + 1)],
                                    start=True, stop=False)
                                nc.tensor.matmul(
                                    pgn, lhsT=encT1[:, t - 1:BT:T],
                                    rhs=whh1[:, 512 * nb:512 * (nb + 1)],
                                    start=False, stop=False)
                                nc.tensor.matmul(
                                    pgn, lhsT=id32r[:],
                                    rhs=gx[:, 512 * nb:512 * (nb + 1)],
                                    start=False, stop=True)
                            else:
                                nc.tensor.matmul(
                                    pgn, lhsT=id32r[:],
                                    rhs=gx[:, 512 * nb:512 * (nb + 1)],
                                    start=True, stop=True)
                        nc.scalar.activation(SG[:, 0:768], pg[:, 0:768],
                                             AF.Sigmoid)
                        nc.scalar.activation(SG[:, 768:1024], pg[:, 768:1024],
                                             AF.Tanh)
                        nc.vector.tensor_tensor(UV[:, 0:256], SG[:, 0:256],
                                                SG[:, 768:1024], op=OP.mult)
                        nc.vector.scalar_tensor_tensor(
                            UV[:, 256:512], in0=SG[:, 256:512],
                            scalar=Mrow[:, t - 1:t] if t > 0 else 1.0,
                            in1=CW[:, 0:256], op0=OP.mult, op1=OP.mult)
                        nc.vector.tensor_tensor(CW[:, 0:256], UV[:, 0:256],
                                                UV[:, 256:512], op=OP.add)
                        nc.scalar.activation(CW[:, 256:512], CW[:, 0:256],
                                             AF.Tanh)
                        h2 = h2p.tile([BS, H], f32, tag="h2", name="h2")
                        nc.vector.scalar_tensor_tensor(
                            h2[:], in0=CW[:, 256:512],
                            scalar=Mrow[:, t:t + 1],
                            in1=SG[:, 512:768], op0=OP.mult, op1=OP.mult)
                        sdma(enc_o[s, :, t, :], h2[:])
                        ptt = ptp.tile([128, 2 * BS], f32, tag="ptt", name="ptt")
                        nc.tensor.transpose(ptt[:, 0:BS], h2[:, 0:128], id32[:])
                        nc.tensor.transpose(ptt[:, BS:2 * BS], h2[:, 128:256],
                                            id32[:])
                        nc.scalar.copy(encT0[:, t:BT:T], ptt[:, 0:BS])
                        nc.scalar.copy(encT1[:, t:BT:T], ptt[:, BS:2 * BS])

                # ---------------- boundary ----------------
                if s < S - 1:
                    with (tc.tile_pool(name=f"ph{s}", bufs=4,
                                       space="PSUM") as php,
                          tc.tile_pool(name=f"pl{s}", bufs=2,
                                       space="PSUM") as plp,
                          tc.tile_pool(name=f"lbsl{s}", bufs=4) as lbp):
                        for nsl in range(8):
                            cs = slice(512 * nsl, 512 * (nsl + 1))
                            hid = [None, None]
                            for mch in range(2):
                                ph = php.tile([128, 512], f32, tag="ph",
                                              name="ph")
                                for k in range(2):
                                    enc_k = (encT0, encT1)[k]
                                    wb1_k = (wb1_0, wb1_1)[k]
                                    nc.tensor.matmul(
                                        ph[:],
                                        lhsT=wb1_k[:, 128 * mch:128 * (mch + 1)],
                                        rhs=enc_k[:, cs],
                                        start=(k == 0), stop=(k == 1))
                                hid[mch] = hidp.tile([128, 512], f32r,
                                                     tag="hid", name="hid")
                                nc.scalar.activation(
                                    hid[mch][:], ph[:], AF.Relu,
                                    bias=bb1_s[:, mch:mch + 1])
                            pl = plp.tile([1, 512], f32, tag="pl", name="pl")
                            for mch in range(2):
                                nc.tensor.matmul(
                                    pl[:], lhsT=wb2_s[:, mch:mch + 1],
                                    rhs=hid[mch][:],
                                    start=(mch == 0), stop=(mch == 1))
                            lbsl = lbp.tile([1, 512], f32, tag="lbsl",
                                            name="lbsl")
                            nc.scalar.activation(lbsl[:], pl[:],
                                                 AF.Identity, bias=bb2_s[:])
                            sdma(blog_o[s:s + 1, cs], lbsl[:])
                            exsl = lbp.tile([1, 512], f32, tag="exsl",
                                            name="exsl")
                            nc.scalar.activation(exsl[:], pl[:], AF.Exp,
                                                 bias=bb2c_s[:])
                            sdma(EX32[4 * nsl:4 * (nsl + 1), :], exsl[:])
                    sdma(EG32[:], egd[s])
                    nc.vector.memset(EX32[:, 0:1], 0.0)
                    nc.vector.tensor_tensor(EX32[:], EX32[:], EG32[:],
                                            op=OP.mult)
                    nc.vector.tensor_reduce(ssum[:], EX32[:], axis=AX.X,
                                            op=OP.add)
                    nc.vector.reciprocal(rsum[:], ssum[:])
                    nc.vector.tensor_scalar_mul(SB32[:], EX32[:], rsum[:])
                else:
                    nc.vector.tensor_scalar(SB32[:], iota32[:], len32c[:],
                                            None, op0=OP.is_equal)
                sdma(bsamp_o[s:s + 1, :], SB32[:])

                if s < S - 1:
                    # cumsum over t per batch row, then mask = exp(sum log)
                    nc.vector.tensor_tensor_scan(
                        EX32[:], data0=ones32[:], data1=SB32[:], initial=0.0,
                        op0=OP.mult, op1=OP.add)
                    nc.scalar.activation(LG32[:], EX32[:], AF.Ln, bias=epsb[0:BS, :])
                    nc.vector.tensor_tensor(LCA32[:], LCA32[:], LG32[:],
                                            op=OP.add)
                    nc.scalar.activation(Mrow[:], LCA32[:], AF.Exp)
                    sdma(mask_o[s:s + 1, :], Mrow[:])

                # ---- readout ----
                sbsh = rows.tile([1, BT], f32, tag="row", name="row")
                nc.vector.memset(sbsh[:], 0.0)
                sdma(sbsh[0:1, :].rearrange("p (b t) -> p b t", t=T)[:, :, 0:T - 1],
                     SB32[:, 1:T])
                with tc.tile_pool(name=f"pb{s}", bufs=2, space="PSUM") as pbp:
                    for nsl in range(8):
                        cs = slice(512 * nsl, 512 * (nsl + 1))
                        pb = pbp.tile([128, 512], f32, tag="pb", name="pb")
                        nc.tensor.matmul(pb[:], lhsT=ones1[:, 0:128],
                                         rhs=sbsh[0:1, cs],
                                         start=True, stop=True)
                        for k in range(2):
                            enc_k = (encT0, encT1)[k]
                            nc.vector.tensor_tensor(PRD[:], enc_k[:, cs],
                                                    pb[:], op=OP.mult)
                            nc.vector.tensor_reduce(
                                rdT[:, k * BS + 4 * nsl:k * BS + 4 * nsl + 4],
                                PRD[:].rearrange("p (b t) -> p b t", t=T),
                                axis=AX.X, op=OP.add)

                # ---- z head + decode ----
                with tc.tile_pool(name=f"pz{s}", bufs=1, space="PSUM") as pzp:
                    pz = pzp.tile([128, 2 * BS], f32, tag="pz", name="pz")
                    for mch in range(2):
                        for k in range(2):
                            wz1_k = (wz1_0, wz1_1)[k]
                            nc.tensor.matmul(
                                pz[:, BS * mch:BS * (mch + 1)],
                                lhsT=wz1_k[:, 128 * mch:128 * (mch + 1)],
                                rhs=rdT[:, BS * k:BS * (k + 1)],
                                start=(k == 0), stop=(k == 1))
                    for mch in range(2):
                        nc.scalar.activation(z1_s[:, BS * mch:BS * (mch + 1)],
                                             pz[:, BS * mch:BS * (mch + 1)],
                                             AF.Relu, bias=bz1_s[:, mch:mch + 1])
                    plz = pzp.tile([128, BS], f32, tag="plz", name="plz")
                    for k in range(2):
                        wz2_k = (wz2_0, wz2_1)[k]
                        nc.tensor.matmul(plz[:], lhsT=wz2_k[:],
                                         rhs=z1_s[:, BS * k:BS * (k + 1)],
                                         start=(k == 0), stop=(k == 1))
                    nc.scalar.activation(ZT[:], plz[:], AF.Identity,
                                         bias=bz2_s[:])
                    sdma(zlog_o[s], ZT[:])
                    # ZT rows: [0:64]=log_var, [64:128]=mu
                    nc.scalar.activation(ZW[:, 0:BS], ZT[0:L, :], AF.Exp,
                                         scale=0.5)
                    nc.vector.tensor_copy(ZW[:, BS:2 * BS], ZT[L:2 * L, :])
                    nc.vector.tensor_tensor(ZW[:, 2 * BS:3 * BS], ZW[:, 0:BS],
                                            eps_s[:, BS * s:BS * (s + 1)],
                                            op=OP.mult)
                    nc.vector.tensor_tensor(szT_s[:], ZW[:, 2 * BS:3 * BS],
                                            ZW[:, BS:2 * BS], op=OP.add)
                    sdma(zsamp_o[s], szT_s[:])

                    pd = pzp.tile([128, 2 * BS], f32, tag="pd", name="pd")
                    for mch in range(2):
                        nc.tensor.matmul(
                            pd[:, BS * mch:BS * (mch + 1)],
                            lhsT=wd1_s[:, 128 * mch:128 * (mch + 1)],
                            rhs=szT_s[:], start=True, stop=True)
                    for mch in range(2):
                        nc.scalar.activation(d1_s[:, BS * mch:BS * (mch + 1)],
                                             pd[:, BS * mch:BS * (mch + 1)],
                                             AF.Relu, bias=bd1_s[:, mch:mch + 1])
                    pp4 = pzp.tile([128, 4 * BS], f32, tag="pp4", name="pp4")
                    for mch in range(4):
                        for k in range(2):
                            wd2_k = (wd2_0, wd2_1)[k]
                            nc.tensor.matmul(
                                pp4[:, BS * mch:BS * (mch + 1)],
                                lhsT=wd2_k[:, 128 * mch:128 * (mch + 1)],
                                rhs=d1_s[:, BS * k:BS * (k + 1)],
                                start=(k == 0), stop=(k == 1))
                    for mch in range(4):
                        nc.scalar.activation(pred_s[:, BS * mch:BS * (mch + 1)],
                                             pp4[:, BS * mch:BS * (mch + 1)],
                                             AF.Identity,
                                             bias=bd2_s[:, mch:mch + 1])
                    ppt = pzp.tile([BS, V], f32, tag="ppt", name="ppt")
                    for mch in range(4):
                        nc.tensor.transpose(ppt[:, 128 * mch:128 * (mch + 1)],
                                            pred_s[:, BS * mch:BS * (mch + 1)],
                                            id128[:])
                    nc.scalar.copy(prow[:], ppt[:])
                for tb in range(8):
                    src = prow[:].rearrange("b (o v) -> b o v", o=1)
                    src = src.broadcast_to([BS, 16, V])
                    sdma(rec_o[s, :, 16 * tb:16 * (tb + 1), :], src)

    return nc


# ----------------------------------------------------------------------------
# Host side
# ----------------------------------------------------------------------------

def _host_noise():
    import jax
    import jax.numpy as jnp
    cpu = jax.local_devices(backend="cpu")[0]
    with jax.default_device(cpu):
        nkey = jax.random.key(42)
        gum, epz = [], []
        for seg in range(S):
            u = jax.random.uniform(jax.random.fold_in(nkey, 2 * seg), (B, T),
                                   jnp.float32)
            gum.append(np.asarray(-jnp.log(EPS - jnp.log(u + EPS))))
            epz.append(np.asarray(jax.random.normal(
                jax.random.fold_in(nkey, 2 * seg + 1), (B, L), jnp.float32)))
    return np.stack(gum), np.stack(epz)


def _reorder(w):
    i, f, g, o = np.split(w, 4, axis=0)
    return np.concatenate([i, f, o, g], axis=0)


def make_in_maps(inputs):
    f32 = np.float32
    if "noise" not in _CACHE:
        _CACHE["noise"] = _host_noise()
    gum, epz = _CACHE["noise"]
    eg = np.exp(gum).astype(f32)

    w_ih = np.asarray(inputs["w_ih"], f32); w_hh = np.asarray(inputs["w_hh"], f32)
    b_ih = np.asarray(inputs["b_ih"], f32); b_hh = np.asarray(inputs["b_hh"], f32)
    wihT = np.ascontiguousarray(_reorder(w_ih).T)
    whhT = np.ascontiguousarray(_reorder(w_hh).T)

    wz2 = np.asarray(inputs["wz2"], f32); bz2 = np.asarray(inputs["bz2"], f32)
    wz2_r = np.concatenate([wz2[L:], wz2[:L]], axis=0)
    bz2_r = np.concatenate([bz2[L:], bz2[:L]])

    def kchunk(a):
        return np.ascontiguousarray(np.stack([a[0:128], a[128:256]]))

    com = dict(
        embw=np.asarray(inputs["embed_w"], f32),
        wihT=kchunk(wihT).astype(f32),
        bihs=_reorder(b_ih)[None].astype(f32),
        bhhs=_reorder(b_hh)[None].astype(f32),
        whhTd=kchunk(whhT).astype(f32),
        wb1Td=kchunk(np.ascontiguousarray(np.asarray(inputs["wb1"], f32).T)),
        wb2d=np.ascontiguousarray(
            np.asarray(inputs["wb2"], f32)[0].reshape(2, 128).T),
        bb1d=np.ascontiguousarray(
            np.asarray(inputs["bb1"], f32).reshape(2, 128).T),
        bb2d=np.asarray(inputs["bb2"], f32).reshape(1, 1),
        wz1Td=kchunk(np.ascontiguousarray(np.asarray(inputs["wz1"], f32).T)),
        bz1d=np.ascontiguousarray(
            np.asarray(inputs["bz1"], f32).reshape(2, 128).T),
        wz2Td=kchunk(np.ascontiguousarray(wz2_r.T)),
        bz2d=np.ascontiguousarray(bz2_r.reshape(128, 1)),
        wd1Td=np.ascontiguousarray(np.asarray(inputs["wd1"], f32).T),
        bd1d=np.ascontiguousarray(
            np.asarray(inputs["bd1"], f32).reshape(2, 128).T),
        wd2Td=kchunk(np.ascontiguousarray(np.asarray(inputs["wd2"], f32).T)),
        bd2d=np.ascontiguousarray(
            np.asarray(inputs["bd2"], f32).reshape(4, 128).T),
        iotad=np.ascontiguousarray(
            np.broadcast_to(np.arange(T, dtype=f32), (BS, T))),
        id32d=np.eye(BS, dtype=f32),
        id32rd=np.eye(BS, dtype=f32),
        id128d=np.eye(128, dtype=f32),
    )

    idx_full = np.asarray(inputs["inputs"]).astype(np.int32)
    len_full = np.asarray(inputs["lengths"]).astype(np.int64)

    in_maps = []
    for core in range(NCORES):
        sh = slice(core * BS, (core + 1) * BS)
        m = dict(com)
        m["idxd"] = np.ascontiguousarray(idx_full[sh])
        m["egd"] = np.ascontiguousarray(eg[:3, sh])
        m["epzd"] = np.ascontiguousarray(
            np.concatenate([epz[s_, sh].T for s_ in range(S)], axis=1))
        m["lend"] = np.ascontiguousarray(
            (len_full[sh] - 1).astype(f32).reshape(BS, 1))
        in_maps.append(m)
    return in_maps


def unshard(results):
    f32 = np.float32
    encs = np.concatenate([r["enc_o"] for r in results], axis=1)
    recs = np.concatenate([r["rec_o"] for r in results], axis=1)
    masks = np.concatenate(
        [r["mask_o"].reshape(S - 1, BS, T) for r in results], axis=1)
    blog = np.concatenate(
        [r["blog_o"].reshape(S - 1, BS, T) for r in results], axis=1)
    blog[:, :, 0] = NEG_INF
    bsamp = np.concatenate(
        [r["bsamp_o"].reshape(S, BS, T) for r in results], axis=1)
    zl = np.concatenate([r["zlog_o"] for r in results], axis=2)
    zlog = np.concatenate([zl[:, L:], zl[:, :L]], axis=1).transpose(0, 2, 1)
    zsamp = np.concatenate([r["zsamp_o"] for r in results],
                           axis=2).transpose(0, 2, 1)
    return (encs.astype(f32), recs.astype(f32), masks.astype(f32),
            blog.astype(f32), bsamp.astype(f32), zlog.astype(f32),
            zsamp.astype(f32))


def kernel(**inputs):
    if "nc" not in _CACHE:
        _CACHE["nc"] = build_bass()
        _split_sync_waits(_CACHE["nc"], 1)
    nc = _CACHE["nc"]
    in_maps = make_in_maps(inputs)
    from concourse.bass_utils import run_bass_kernel_spmd
    res = run_bass_kernel_spmd(nc, in_maps, list(range(NCORES)),
                               trace=bool(os.environ.get("KTRACE")))
    _CACHE["last"] = res
    return unshard(res.results)


# revision 11
# speedup vs baseline: 1.0099x; 1.0099x over previous
"""Trainium2 Bass kernel for nn_CompILE (CompILE forward), 8-core data parallel.

Sharding: batch B=256 split across 8 NeuronCores (32 each); weights replicated.
Per core:
  P0: transpose embed_w on-device; build gate-embedding table
      EW[v, :] = w_ih_r @ embed_w[v] + (b_ih + b_hh) (gate order i,f,o,g) in DRAM.
  LSTM (4 segments x 128 sequential steps): gates = h @ w_hh_r.T + EW[idx_t]
      (PE matmuls, fp32r moving operand; EW row fetched by indirect DMA and
      added via identity matmul); sigmoid/tanh on ACT; cell update on DVE;
      PE-transposed h kept in SBUF as encT [H, b*T+t] for the boundary heads.
  Boundary: hid = relu(wb1 @ encT + b); lb = wb2 . hid; gumbel softmax as
      exp(lb - C)*exp(gumbel) normalized per row (host ships exp(gumbel),
      threefry seed 42, matching the reference); cumsum via tensor_tensor_scan;
      mask = exp(accumulated log cumsum); readout/z/decode in transposed form.
Host side only reorders/shards/transposes parameter layouts, precomputes the
fixed-seed noise constants, and reassembles device outputs.
"""
import os
import sys
import numpy as np

if "/opt/trn_rl_repo" not in sys.path:
    sys.path.insert(0, "/opt/trn_rl_repo")

EPS = 1e-17
NEG_INF = -1e30
B, T, V, H, L, S = 256, 128, 512, 256, 64, 4
BS = 32
BT = BS * T  # per-core rows, b-major: col = b*T + t
CSOFT = 10.0
NCORES = 8

_CACHE = {}


def _split_sync_waits(nc, limit=1):
    """walrus codegen in this build rejects >1 sync wait per instruction;
    hoist excess waits onto same-engine no-ops inserted just before."""
    import bass_rust
    import concourse.mybir as mybir
    n = 0
    for fn in nc.m.functions:
        for bb in fn.blocks:
            out = []
            changed = False
            for inst in bb.instructions:
                si = getattr(inst, "sync_info", None)
                ow = list(si.on_wait) if (si is not None and si.on_wait) else []
                if len(ow) > limit:
                    keep = ow[-limit:]
                    extra = ow[:-limit]
                    for j in range(0, len(extra), limit):
                        nop = bass_rust.InstNoOp(
                            name=f"I-wsplit-{n}", ins=[], outs=[])
                        n += 1
                        nop.engine = inst.engine
                        nop.sync_info = mybir.SyncInfo(
                            on_wait=extra[j:j + limit], on_update=[])
                        out.append(nop)
                    si.on_wait = keep
                    changed = True
                out.append(inst)
            if changed:
                try:
                    bb.instructions = out
                except Exception:
                    bb.instructions.clear()
                    for x in out:
                        bb.instructions.append(x)
    return n


def build_bass():
    import concourse.bass as bass
    import concourse.mybir as mybir
    import concourse.tile as tile
    from concourse.bass import IndirectOffsetOnAxis

    f32 = mybir.dt.float32
    f32r = mybir.dt.float32r
    i32 = mybir.dt.int32
    AF = mybir.ActivationFunctionType
    OP = mybir.AluOpType
    AX = mybir.AxisListType

    nc = bass.Bass("TRN2", target_bir_lowering=False, debug=False,
                   num_devices=NCORES)

    def din(name, shape, dtype=f32):
        return nc.dram_tensor(name, shape, dtype, kind="ExternalInput")

    def dout(name, shape, dtype=f32):
        return nc.dram_tensor(name, shape, dtype, kind="ExternalOutput")

    embw = din("embw", [V, H])
    wihT = din("wihT", [2, 128, 4 * H], f32r)
    bihs = din("bihs", [1, 4 * H])
    bhhs = din("bhhs", [1, 4 * H])
    whhTd = din("whhTd", [2, 128, 4 * H], f32r)
    wb1Td = din("wb1Td", [2, 128, H], f32r)
    wb2d = din("wb2d", [128, 2], f32r)
    bb1d = din("bb1d", [128, 2])
    bb2d = din("bb2d", [1, 1])
    wz1Td = din("wz1Td", [2, 128, H])
    bz1d = din("bz1d", [128, 2])
    wz2Td = din("wz2Td", [2, 128, 2 * L])   # rows reordered to [log_var; mu]
    bz2d = din("bz2d", [128, 1])
    wd1Td = din("wd1Td", [L, H])
    bd1d = din("bd1d", [128, 2])
    wd2Td = din("wd2Td", [2, 128, V])
    bd2d = din("bd2d", [128, 4])
    idxd = din("idxd", [BS, T], i32)
    egd = din("egd", [3, BS, T])
    epzd = din("epzd", [L, 4 * BS])
    lend = din("lend", [BS, 1])
    iotad = din("iotad", [BS, T])
    id32d = din("id32d", [BS, BS])
    id32rd = din("id32rd", [BS, BS], f32r)
    id128d = din("id128d", [128, 128])

    ew_d = nc.dram_tensor("ew_d", [V, 4 * H], f32r)

    enc_o = dout("enc_o", [S, BS, T, H])
    rec_o = dout("rec_o", [S, BS, T, V])
    mask_o = dout("mask_o", [S - 1, BT])
    blog_o = dout("blog_o", [S - 1, BT])
    bsamp_o = dout("bsamp_o", [S, BT])
    zlog_o = dout("zlog_o", [S, 2 * L, BS])
    zsamp_o = dout("zsamp_o", [S, L, BS])

    def r(ap):
        return ap.bitcast(f32r)

    with tile.TileContext(nc) as tc:
      with tc.tile_pool(name="persist", bufs=1) as pp:
        def ptile(tag, shape, dtype=f32):
            return pp.tile(shape, dtype, tag=tag, name=tag)

        whh0 = ptile("whh0", [128, 4 * H], f32r); whh1 = ptile("whh1", [128, 4 * H], f32r)
        encT0 = ptile("encT0", [128, BT], f32r); encT1 = ptile("encT1", [128, BT], f32r)
        SG = ptile("SG", [BS, 1024])    # sig_ifo 0:768 | tanh_g 768:1024
        CW = ptile("CW", [BS, 512])     # cu 0:256 | tanh(cu) 256:512
        UV = ptile("UV", [BS, 512])
        Mrow = ptile("Mrow", [BS, T])
        idx_s = ptile("idx_s", [BS, T], i32)
        id32 = ptile("id32", [BS, BS])
        id32r = ptile("id32r", [BS, BS], f32r)
        id128 = ptile("id128", [128, 128])
        eps_s = ptile("eps_s", [L, 4 * BS])
        wb1_0 = ptile("wb1_0", [128, H], f32r); wb1_1 = ptile("wb1_1", [128, H], f32r)
        wb2_s = ptile("wb2_s", [128, 2], f32r); bb1_s = ptile("bb1_s", [128, 2])
        bb2_s = ptile("bb2_s", [1, 1]); bb2c_s = ptile("bb2c_s", [1, 1])
        wz1_0 = ptile("wz1_0", [128, H]); wz1_1 = ptile("wz1_1", [128, H])
        bz1_s = ptile("bz1_s", [128, 2])
        wz2_0 = ptile("wz2_0", [128, 2 * L]); wz2_1 = ptile("wz2_1", [128, 2 * L])
        bz2_s = ptile("bz2_s", [128, 1])
        wd1_s = ptile("wd1_s", [L, H]); bd1_s = ptile("bd1_s", [128, 2])
        wd2_0 = ptile("wd2_0", [128, V]); wd2_1 = ptile("wd2_1", [128, V])
        bd2_s = ptile("bd2_s", [128, 4])
        LCA32 = ptile("LCA32", [BS, T]); LG32 = ptile("LG32", [BS, T])
        EG32 = ptile("EG32", [BS, T]); ones32 = ptile("ones32", [BS, T])
        iota32 = ptile("iota32", [BS, T]); len32c = ptile("len32c", [BS, 1])
        ones1 = ptile("ones1", [1, 128])
        epsb = ptile("epsb", [128, 1])
        EX32 = ptile("EX32", [BS, T]); SB32 = ptile("SB32", [BS, T])
        ssum = ptile("ssum", [BS, 1]); rsum = ptile("rsum", [BS, 1])
        PRD = ptile("PRD", [128, 512])
        rdT = ptile("rdT", [128, 2 * BS])
        ZT = ptile("ZT", [128, BS])
        ZW = ptile("ZW", [L, 3 * BS])
        szT_s = ptile("szT_s", [L, BS])
        z1_s = ptile("z1_s", [128, 2 * BS])
        d1_s = ptile("d1_s", [128, 2 * BS])
        pred_s = ptile("pred_s", [128, 4 * BS])
        prow = ptile("prow", [BS, V])

        sdma = nc.sync.dma_start
        for dst, src in [
            (whh0[:], whhTd[0]), (whh1[:], whhTd[1]),
            (idx_s[:], idxd[:, :]), (id32[:], id32d[:, :]), (id32r[:], id32rd[:, :]),
            (id128[:], id128d[:, :]), (eps_s[:], epzd[:, :]),
            (wb1_0[:], wb1Td[0]), (wb1_1[:], wb1Td[1]),
            (wb2_s[:], wb2d[:, :]), (bb1_s[:], bb1d[:, :]),
            (bb2_s[:], bb2d[:, :]),
            (wz1_0[:], wz1Td[0]), (wz1_1[:], wz1Td[1]), (bz1_s[:], bz1d[:, :]),
            (wz2_0[:], wz2Td[0]), (wz2_1[:], wz2Td[1]), (bz2_s[:], bz2d[:, :]),
            (wd1_s[:], wd1Td[:, :]), (bd1_s[:], bd1d[:, :]),
            (wd2_0[:], wd2Td[0]), (wd2_1[:], wd2Td[1]), (bd2_s[:], bd2d[:, :]),
            (iota32[:], iotad[:, :]), (len32c[:], lend[:, :]),
        ]:
            sdma(dst, src)
        nc.vector.memset(ones1[:], 1.0)
        nc.vector.memset(epsb[:], EPS)
        nc.vector.memset(ones32[:], 1.0)
        nc.vector.tensor_scalar_add(bb2c_s[:], bb2_s[:], -CSOFT)

        # ---------------- P0: EW table ----------------
        ew_dmas = []
        with (tc.tile_pool(name="p0sb", bufs=2) as p0sb,
              tc.tile_pool(name="p0ps", bufs=2, space="PSUM") as p0ps,
              tc.tile_pool(name="p0ps2", bufs=2, space="PSUM") as p0ps2):
            embT = [p0sb.tile([128, V], f32r, tag="embT0", name="embT0"),
                    p0sb.tile([128, V], f32r, tag="embT1", name="embT1")]
            wihs = [p0sb.tile([128, 4 * H], f32r, tag="wihs0", name="wihs0"),
                    p0sb.tile([128, 4 * H], f32r, tag="wihs1", name="wihs1")]
            sdma(wihs[0][:], wihT[0]); sdma(wihs[1][:], wihT[1])
            bsum = p0sb.tile([1, 4 * H], f32, tag="bsum", name="bsum")
            bih_t = p0sb.tile([1, 4 * H], f32, tag="bih_t", name="bih_t")
            bhh_t = p0sb.tile([1, 4 * H], f32, tag="bhh_t", name="bhh_t")
            sdma(bih_t[:], bihs[:, :]); sdma(bhh_t[:], bhhs[:, :])
            nc.vector.tensor_tensor(bsum[:], bih_t[:], bhh_t[:], op=OP.add)

            for vc in range(4):
                et = p0sb.tile([128, H], f32, tag="et", name="et")
                sdma(et[:], embw[128 * vc:128 * (vc + 1), :])
                for hc in range(2):
                    pt0 = p0ps.tile([128, 128], f32, tag="p0t", name="p0t")
                    nc.tensor.transpose(pt0[:], et[:, 128 * hc:128 * (hc + 1)],
                                        id128[:])
                    nc.scalar.copy(embT[hc][:, 128 * vc:128 * (vc + 1)], pt0[:])

            for vc in range(4):
                ewst = p0sb.tile([128, 4 * H], f32r, tag="ewst", name="ewst")
                for nb in range(2):
                    pe = p0ps2.tile([128, 512], f32, tag="p0e", name="p0e")
                    for k in range(2):
                        nc.tensor.matmul(
                            pe[:], lhsT=embT[k][:, 128 * vc:128 * (vc + 1)],
                            rhs=wihs[k][:, 512 * nb:512 * (nb + 1)],
                            start=(k == 0), stop=False)
                    nc.tensor.matmul(pe[:], lhsT=ones1[:, 0:128],
                                     rhs=bsum[:, 512 * nb:512 * (nb + 1)],
                                     start=False, stop=True)
                    nc.scalar.copy(ewst[:, 512 * nb:512 * (nb + 1)], pe[:])
                ew_dmas.append(sdma(ew_d[128 * vc:128 * (vc + 1), :], ewst[:]))

        # ---------------- segments ----------------
        with (tc.tile_pool(name="gx", bufs=6) as gxp,
              tc.tile_pool(name="h2", bufs=3) as h2p,
              tc.tile_pool(name="rows", bufs=4) as rows,
              tc.tile_pool(name="hidp", bufs=3) as hidp):

            for s in range(S):
                if s == 0:
                    nc.vector.memset(Mrow[:], 1.0)
                    nc.vector.memset(LCA32[:], 0.0)
                nc.vector.memset(CW[:, 0:256], 0.0)

                with (tc.tile_pool(name=f"pg{s}", bufs=2, space="PSUM") as pgp,
                      tc.tile_pool(name=f"pt{s}", bufs=2, space="PSUM") as ptp,
                      tc.tile_pool(name=f"pdum{s}", bufs=1, space="PSUM") as pdp):
                    pdum = pdp.tile([BS, 64], f32, tag="pdum", name="pdum")
                    for t in range(T):
                        gx = gxp.tile([BS, 4 * H], f32r, tag="gx", name="gx")
                        gi = nc.gpsimd.indirect_dma_start(
                            out=gx[:], out_offset=None,
                            in_=ew_d[:, :],
                            in_offset=IndirectOffsetOnAxis(
                                ap=idx_s[:, t:t + 1], axis=0))
                        if s == 0 and t == 0:
                            for d in ew_dmas:
                                tile.add_dep_helper(gi.ins, d.ins,
                                                    reason="gather after EW")
                        pg = pgp.tile([BS, 4 * H], f32, tag="pg", name="pg")
                        for nb in range(2):
                            pgn = pg[:, 512 * nb:512 * (nb + 1)]
                            if t > 0:
                                nc.tensor.matmul(
                                    pgn, lhsT=encT0[:, t - 1:BT:T],
                                    rhs=whh0[:, 512 * nb:512 * (nb + 1)],
                                    start=True, stop=False)
                                nc.tensor.matmul(
                                    pgn, lhsT=encT1[:, t - 1:BT:T],
                                    rhs=whh1[:, 512 * nb:512 * (nb + 1)],
                                    start=False, stop=False)
                                nc.tensor.matmul(
                                    pgn, lhsT=id32r[:],
                                    rhs=gx[:, 512 * nb:512 * (nb + 1)],
                                    start=False, stop=True)
                            else:
                                nc.tensor.matmul(
                                    pgn, lhsT=id32r[:],
                                    rhs=gx[:, 512 * nb:512 * (nb + 1)],
                                    start=True, stop=True)
                        nc.scalar.activation(SG[:, 0:768], pg[:, 0:768],
                                             AF.Sigmoid)
                        nc.scalar.activation(SG[:, 768:1024], pg[:, 768:1024],
                                             AF.Tanh)
                        nc.vector.tensor_tensor(UV[:, 0:256], SG[:, 0:256],
                                                SG[:, 768:1024], op=OP.mult)
                        nc.vector.scalar_tensor_tensor(
                            UV[:, 256:512], in0=SG[:, 256:512],
                            scalar=Mrow[:, t - 1:t] if t > 0 else 1.0,
                            in1=CW[:, 0:256], op0=OP.mult, op1=OP.mult)
                        nc.tensor.matmul(pdum[:], lhsT=id32[:],
                                         rhs=UV[:, 0:64], start=True, stop=True)
                        nc.vector.tensor_tensor(CW[:, 0:256], UV[:, 0:256],
                                                UV[:, 256:512], op=OP.add)
                        nc.scalar.activation(CW[:, 256:512], CW[:, 0:256],
                                             AF.Tanh)
                        nc.tensor.matmul(pdum[:], lhsT=id32[:],
                                         rhs=CW[:, 256:320], start=True,
                                         stop=True)
                        h2 = h2p.tile([BS, H], f32, tag="h2", name="h2")
                        nc.vector.scalar_tensor_tensor(
                            h2[:], in0=CW[:, 256:512],
                            scalar=Mrow[:, t:t + 1],
                            in1=SG[:, 512:768], op0=OP.mult, op1=OP.mult)
                        sdma(enc_o[s, :, t, :], h2[:])
                        ptt = ptp.tile([128, 2 * BS], f32, tag="ptt", name="ptt")
                        nc.tensor.transpose(ptt[:, 0:BS], h2[:, 0:128], id32[:])
                        nc.tensor.transpose(ptt[:, BS:2 * BS], h2[:, 128:256],
                                            id32[:])
                        nc.scalar.copy(encT0[:, t:BT:T], ptt[:, 0:BS])
                        nc.vector.tensor_copy(encT1[:, t:BT:T], ptt[:, BS:2 * BS])

                # ---------------- boundary ----------------
                if s < S - 1:
                    with (tc.tile_pool(name=f"ph{s}", bufs=4,
                                       space="PSUM") as php,
                          tc.tile_pool(name=f"pl{s}", bufs=2,
                                       space="PSUM") as plp,
                          tc.tile_pool(name=f"lbsl{s}", bufs=4) as lbp):
                        for nsl in range(8):
                            cs = slice(512 * nsl, 512 * (nsl + 1))
                            hid = [None, None]
                            for mch in range(2):
                                ph = php.tile([128, 512], f32, tag="ph",
                                              name="ph")
                                for k in range(2):
                                    enc_k = (encT0, encT1)[k]
                                    wb1_k = (wb1_0, wb1_1)[k]
                                    nc.tensor.matmul(
                                        ph[:],
                                        lhsT=wb1_k[:, 128 * mch:128 * (mch + 1)],
                                        rhs=enc_k[:, cs],
                                        start=(k == 0), stop=(k == 1))
                                hid[mch] = hidp.tile([128, 512], f32r,
                                                     tag="hid", name="hid")
                                nc.scalar.activation(
                                    hid[mch][:], ph[:], AF.Relu,
                                    bias=bb1_s[:, mch:mch + 1])
                            pl = plp.tile([1, 512], f32, tag="pl", name="pl")
                            for mch in range(2):
                                nc.tensor.matmul(
                                    pl[:], lhsT=wb2_s[:, mch:mch + 1],
                                    rhs=hid[mch][:],
                                    start=(mch == 0), stop=(mch == 1))
                            lbsl = lbp.tile([1, 512], f32, tag="lbsl",
                                            name="lbsl")
                            nc.scalar.activation(lbsl[:], pl[:],
                                                 AF.Identity, bias=bb2_s[:])
                            sdma(blog_o[s:s + 1, cs], lbsl[:])
                            exsl = lbp.tile([1, 512], f32, tag="exsl",
                                            name="exsl")
                            nc.scalar.activation(exsl[:], pl[:], AF.Exp,
                                                 bias=bb2c_s[:])
                            sdma(EX32[4 * nsl:4 * (nsl + 1), :], exsl[:])
                    sdma(EG32[:], egd[s])
                    nc.vector.memset(EX32[:, 0:1], 0.0)
                    nc.vector.tensor_tensor(EX32[:], EX32[:], EG32[:],
                                            op=OP.mult)
                    nc.vector.tensor_reduce(ssum[:], EX32[:], axis=AX.X,
                                            op=OP.add)
                    nc.vector.reciprocal(rsum[:], ssum[:])
                    nc.vector.tensor_scalar_mul(SB32[:], EX32[:], rsum[:])
                else:
                    nc.vector.tensor_scalar(SB32[:], iota32[:], len32c[:],
                                            None, op0=OP.is_equal)
                sdma(bsamp_o[s:s + 1, :], SB32[:])

                if s < S - 1:
                    # cumsum over t per batch row, then mask = exp(sum log)
                    nc.vector.tensor_tensor_scan(
                        EX32[:], data0=ones32[:], data1=SB32[:], initial=0.0,
                        op0=OP.mult, op1=OP.add)
                    nc.scalar.activation(LG32[:], EX32[:], AF.Ln, bias=epsb[0:BS, :])
                    nc.vector.tensor_tensor(LCA32[:], LCA32[:], LG32[:],
                                            op=OP.add)
                    nc.scalar.activation(Mrow[:], LCA32[:], AF.Exp)
                    sdma(mask_o[s:s + 1, :], Mrow[:])

                # ---- readout ----
                sbsh = rows.tile([1, BT], f32, tag="row", name="row")
                nc.vector.memset(sbsh[:], 0.0)
                sdma(sbsh[0:1, :].rearrange("p (b t) -> p b t", t=T)[:, :, 0:T - 1],
                     SB32[:, 1:T])
                with tc.tile_pool(name=f"pb{s}", bufs=2, space="PSUM") as pbp:
                    for nsl in range(8):
                        cs = slice(512 * nsl, 512 * (nsl + 1))
                        pb = pbp.tile([128, 512], f32, tag="pb", name="pb")
                        nc.tensor.matmul(pb[:], lhsT=ones1[:, 0:128],
                                         rhs=sbsh[0:1, cs],
                                         start=True, stop=True)
                        for k in range(2):
                            enc_k = (encT0, encT1)[k]
                            nc.vector.tensor_tensor(PRD[:], enc_k[:, cs],
                                                    pb[:], op=OP.mult)
                            nc.vector.tensor_reduce(
                                rdT[:, k * BS + 4 * nsl:k * BS + 4 * nsl + 4],
                                PRD[:].rearrange("p (b t) -> p b t", t=T),
                                axis=AX.X, op=OP.add)

                # ---- z head + decode ----
                with tc.tile_pool(name=f"pz{s}", bufs=1, space="PSUM") as pzp:
                    pz = pzp.tile([128, 2 * BS], f32, tag="pz", name="pz")
                    for mch in range(2):
                        for k in range(2):
                            wz1_k = (wz1_0, wz1_1)[k]
                            nc.tensor.matmul(
                                pz[:, BS * mch:BS * (mch + 1)],
                                lhsT=wz1_k[:, 128 * mch:128 * (mch + 1)],
                                rhs=rdT[:, BS * k:BS * (k + 1)],
                                start=(k == 0), stop=(k == 1))
                    for mch in range(2):
                        nc.scalar.activation(z1_s[:, BS * mch:BS * (mch + 1)],
                                             pz[:, BS * mch:BS * (mch + 1)],
                                             AF.Relu, bias=bz1_s[:, mch:mch + 1])
                    plz = pzp.tile([128, BS], f32, tag="plz", name="plz")
                    for k in range(2):
                        wz2_k = (wz2_0, wz2_1)[k]
                        nc.tensor.matmul(plz[:], lhsT=wz2_k[:],
                                         rhs=z1_s[:, BS * k:BS * (k + 1)],
                                         start=(k == 0), stop=(k == 1))
                    nc.scalar.activation(ZT[:], plz[:], AF.Identity,
                                         bias=bz2_s[:])
                    sdma(zlog_o[s], ZT[:])
                    # ZT rows: [0:64]=log_var, [64:128]=mu
                    nc.scalar.activation(ZW[:, 0:BS], ZT[0:L, :], AF.Exp,
                                         scale=0.5)
                    nc.vector.tensor_copy(ZW[:, BS:2 * BS], ZT[L:2 * L, :])
                    nc.vector.tensor_tensor(ZW[:, 2 * BS:3 * BS], ZW[:, 0:BS],
                                            eps_s[:, BS * s:BS * (s + 1)],
                                            op=OP.mult)
                    nc.vector.tensor_tensor(szT_s[:], ZW[:, 2 * BS:3 * BS],
                                            ZW[:, BS:2 * BS], op=OP.add)
                    sdma(zsamp_o[s], szT_s[:])

                    pd = pzp.tile([128, 2 * BS], f32, tag="pd", name="pd")
                    for mch in range(2):
                        nc.tensor.matmul(
                            pd[:, BS * mch:BS * (mch + 1)],
                            lhsT=wd1_s[:, 128 * mch:128 * (mch + 1)],
                            rhs=szT_s[:], start=True, stop=True)
                    for mch in range(2):
                        nc.scalar.activation(d1_s[:, BS * mch:BS * (mch + 1)],
                                             pd[:, BS * mch:BS * (mch + 1)],
                                             AF.Relu, bias=bd1_s[:, mch:mch + 1])
                    pp4 = pzp.tile([128, 4 * BS], f32, tag="pp4", name="pp4")
                    for mch in range(4):
                        for k in range(2):
                            wd2_k = (wd2_0, wd2_1)[k]
                            nc.tensor.matmul(
                                pp4[:, BS * mch:BS * (mch + 1)],
                                lhsT=wd2_k[:, 128 * mch:128 * (mch + 1)],
                                rhs=d1_s[:, BS * k:BS * (k + 1)],
                                start=(k == 0), stop=(k == 1))
                    for mch in range(4):
                        nc.scalar.activation(pred_s[:, BS * mch:BS * (mch + 1)],
                                             pp4[:, BS * mch:BS * (mch + 1)],
                                             AF.Identity,
                                             bias=bd2_s[:, mch:mch + 1])
                    ppt = pzp.tile([BS, V], f32, tag="ppt", name="ppt")
                    for mch in range(4):
                        nc.tensor.transpose(ppt[:, 128 * mch:128 * (mch + 1)],
                                            pred_s[:, BS * mch:BS * (mch + 1)],
                                            id128[:])
                    nc.scalar.copy(prow[:], ppt[:])
                for tb in range(8):
                    src = prow[:].rearrange("b (o v) -> b o v", o=1)
                    src = src.broadcast_to([BS, 16, V])
                    sdma(rec_o[s, :, 16 * tb:16 * (tb + 1), :], src)

    return nc


# ----------------------------------------------------------------------------
# Host side
# ----------------------------------------------------------------------------

def _host_noise():
    import jax
    import jax.numpy as jnp
    cpu = jax.local_devices(backend="cpu")[0]
    with jax.default_device(cpu):
        nkey = jax.random.key(42)
        gum, epz = [], []
        for seg in range(S):
            u = jax.random.uniform(jax.random.fold_in(nkey, 2 * seg), (B, T),
                                   jnp.float32)
            gum.append(np.asarray(-jnp.log(EPS - jnp.log(u + EPS))))
            epz.append(np.asarray(jax.random.normal(
                jax.random.fold_in(nkey, 2 * seg + 1), (B, L), jnp.float32)))
    return np.stack(gum), np.stack(epz)


def _reorder(w):
    i, f, g, o = np.split(w, 4, axis=0)
    return np.concatenate([i, f, o, g], axis=0)


def make_in_maps(inputs):
    f32 = np.float32
    if "noise" not in _CACHE:
        _CACHE["noise"] = _host_noise()
    gum, epz = _CACHE["noise"]
    eg = np.exp(gum).astype(f32)

    w_ih = np.asarray(inputs["w_ih"], f32); w_hh = np.asarray(inputs["w_hh"], f32)
    b_ih = np.asarray(inputs["b_ih"], f32); b_hh = np.asarray(inputs["b_hh"], f32)
    wihT = np.ascontiguousarray(_reorder(w_ih).T)
    whhT = np.ascontiguousarray(_reorder(w_hh).T)

    wz2 = np.asarray(inputs["wz2"], f32); bz2 = np.asarray(inputs["bz2"], f32)
    wz2_r = np.concatenate([wz2[L:], wz2[:L]], axis=0)
    bz2_r = np.concatenate([bz2[L:], bz2[:L]])

    def kchunk(a):
        return np.ascontiguousarray(np.stack([a[0:128], a[128:256]]))

    com = dict(
        embw=np.asarray(inputs["embed_w"], f32),
        wihT=kchunk(wihT).astype(f32),
        bihs=_reorder(b_ih)[None].astype(f32),
        bhhs=_reorder(b_hh)[None].astype(f32),
        whhTd=kchunk(whhT).astype(f32),
        wb1Td=kchunk(np.ascontiguousarray(np.asarray(inputs["wb1"], f32).T)),
        wb2d=np.ascontiguousarray(
            np.asarray(inputs["wb2"], f32)[0].reshape(2, 128).T),
        bb1d=np.ascontiguousarray(
            np.asarray(inputs["bb1"], f32).reshape(2, 128).T),
        bb2d=np.asarray(inputs["bb2"], f32).reshape(1, 1),
        wz1Td=kchunk(np.ascontiguousarray(np.asarray(inputs["wz1"], f32).T)),
        bz1d=np.ascontiguousarray(
            np.asarray(inputs["bz1"], f32).reshape(2, 128).T),
        wz2Td=kchunk(np.ascontiguousarray(wz2_r.T)),
        bz2d=np.ascontiguousarray(bz2_r.reshape(128, 1)),
        wd1Td=np.ascontiguousarray(np.asarray(inputs["wd1"], f32).T),
        bd1d=np.ascontiguousarray(
            np.asarray(inputs["bd1"], f32).reshape(2, 128).T),
        wd2Td=kchunk(np.ascontiguousarray(np.asarray(inputs["wd2"], f32).T)),
        bd2d=np.ascontiguousarray(
            np.asarray(inputs["bd2"], f32).reshape(4, 128).T),
        iotad=np.ascontiguousarray(
            np.broadcast_to(np.arange(T, dtype=f32), (BS, T))),
        id32d=np.eye(BS, dtype=f32),
        id32rd=np.eye(BS, dtype=f32),
        id128d=np.eye(128, dtype=f32),
    )

    idx_full = np.asarray(inputs["inputs"]).astype(np.int32)
    len_full = np.asarray(inputs["lengths"]).astype(np.int64)

    in_maps = []
    for core in range(NCORES):
        sh = slice(core * BS, (core + 1) * BS)
        m = dict(com)
        m["idxd"] = np.ascontiguousarray(idx_full[sh])
        m["egd"] = np.ascontiguousarray(eg[:3, sh])
        m["epzd"] = np.ascontiguousarray(
            np.concatenate([epz[s_, sh].T for s_ in range(S)], axis=1))
        m["lend"] = np.ascontiguousarray(
            (len_full[sh] - 1).astype(f32).reshape(BS, 1))
        in_maps.append(m)
    return in_maps


def unshard(results):
    f32 = np.float32
    encs = np.concatenate([r["enc_o"] for r in results], axis=1)
    recs = np.concatenate([r["rec_o"] for r in results], axis=1)
    masks = np.concatenate(
        [r["mask_o"].reshape(S - 1, BS, T) for r in results], axis=1)
    blog = np.concatenate(
        [r["blog_o"].reshape(S - 1, BS, T) for r in results], axis=1)
    blog[:, :, 0] = NEG_INF
    bsamp = np.concatenate(
        [r["bsamp_o"].reshape(S, BS, T) for r in results], axis=1)
    zl = np.concatenate([r["zlog_o"] for r in results], axis=2)
    zlog = np.concatenate([zl[:, L:], zl[:, :L]], axis=1).transpose(0, 2, 1)
    zsamp = np.concatenate([r["zsamp_o"] for r in results],
                           axis=2).transpose(0, 2, 1)
    return (encs.astype(f32), recs.astype(f32), masks.astype(f32),
            blog.astype(f32), bsamp.astype(f32), zlog.astype(f32),
            zsamp.astype(f32))


def kernel(**inputs):
    if "nc" not in _CACHE:
        _CACHE["nc"] = build_bass()
        _split_sync_waits(_CACHE["nc"], 1)
    nc = _CACHE["nc"]
    in_maps = make_in_maps(inputs)
    from concourse.bass_utils import run_bass_kernel_spmd
    res = run_bass_kernel_spmd(nc, in_maps, list(range(NCORES)),
                               trace=bool(os.environ.get("KTRACE")))
    _CACHE["last"] = res
    return unshard(res.results)


# revision 14
# speedup vs baseline: 1.0355x; 1.0253x over previous
"""Trainium2 Bass kernel for nn_CompILE (CompILE forward), 8-core data parallel.

Sharding: batch B=256 split across 8 NeuronCores (32 each); weights replicated.
Per core:
  P0: transpose embed_w on-device; build gate-embedding table
      EW[v, :] = w_ih_r @ embed_w[v] + (b_ih + b_hh) (gate order i,f,o,g) in DRAM.
  LSTM (4 segments x 128 sequential steps): gates = h @ w_hh_r.T + EW[idx_t]
      (PE matmuls, fp32r moving operand; EW row fetched by indirect DMA and
      added via identity matmul); sigmoid/tanh on ACT; cell update on DVE;
      PE-transposed h kept in SBUF as encT [H, b*T+t] for the boundary heads.
  Boundary: hid = relu(wb1 @ encT + b); lb = wb2 . hid; gumbel softmax as
      exp(lb - C)*exp(gumbel) normalized per row (host ships exp(gumbel),
      threefry seed 42, matching the reference); cumsum via tensor_tensor_scan;
      mask = exp(accumulated log cumsum); readout/z/decode in transposed form.
Host side only reorders/shards/transposes parameter layouts, precomputes the
fixed-seed noise constants, and reassembles device outputs.
"""
import os
import sys
import numpy as np

if "/opt/trn_rl_repo" not in sys.path:
    sys.path.insert(0, "/opt/trn_rl_repo")

EPS = 1e-17
NEG_INF = -1e30
B, T, V, H, L, S = 256, 128, 512, 256, 64, 4
BS = 32
BT = BS * T  # per-core rows, b-major: col = b*T + t
CSOFT = 10.0
NCORES = 8

_CACHE = {}


def _split_sync_waits(nc, limit=1):
    """walrus codegen in this build rejects >1 sync wait per instruction;
    hoist excess waits onto same-engine no-ops inserted just before."""
    import bass_rust
    import concourse.mybir as mybir
    n = 0
    for fn in nc.m.functions:
        for bb in fn.blocks:
            out = []
            changed = False
            for inst in bb.instructions:
                si = getattr(inst, "sync_info", None)
                ow = list(si.on_wait) if (si is not None and si.on_wait) else []
                if len(ow) > limit:
                    keep = ow[-limit:]
                    extra = ow[:-limit]
                    for j in range(0, len(extra), limit):
                        nop = bass_rust.InstNoOp(
                            name=f"I-wsplit-{n}", ins=[], outs=[])
                        n += 1
                        nop.engine = inst.engine
                        nop.sync_info = mybir.SyncInfo(
                            on_wait=extra[j:j + limit], on_update=[])
                        out.append(nop)
                    si.on_wait = keep
                    changed = True
                out.append(inst)
            if changed:
                try:
                    bb.instructions = out
                except Exception:
                    bb.instructions.clear()
                    for x in out:
                        bb.instructions.append(x)
    return n


def build_bass():
    import concourse.bass as bass
    import concourse.mybir as mybir
    import concourse.tile as tile
    from concourse.bass import IndirectOffsetOnAxis

    f32 = mybir.dt.float32
    f32r = mybir.dt.float32r
    i32 = mybir.dt.int32
    AF = mybir.ActivationFunctionType
    OP = mybir.AluOpType
    AX = mybir.AxisListType

    nc = bass.Bass("TRN2", target_bir_lowering=False, debug=False,
                   num_devices=NCORES)

    def din(name, shape, dtype=f32):
        return nc.dram_tensor(name, shape, dtype, kind="ExternalInput")

    def dout(name, shape, dtype=f32):
        return nc.dram_tensor(name, shape, dtype, kind="ExternalOutput")

    embw = din("embw", [V, H])
    wihT = din("wihT", [2, 128, 4 * H], f32r)
    bihs = din("bihs", [1, 4 * H])
    bhhs = din("bhhs", [1, 4 * H])
    whhTd = din("whhTd", [2, 128, 4 * H], f32r)
    wb1Td = din("wb1Td", [2, 128, H], f32r)
    wb2d = din("wb2d", [128, 2], f32r)
    bb1d = din("bb1d", [128, 2])
    bb2d = din("bb2d", [1, 1])
    wz1Td = din("wz1Td", [2, 128, H])
    bz1d = din("bz1d", [128, 2])
    wz2Td = din("wz2Td", [2, 128, 2 * L])   # rows reordered to [log_var; mu]
    bz2d = din("bz2d", [128, 1])
    wd1Td = din("wd1Td", [L, H])
    bd1d = din("bd1d", [128, 2])
    wd2Td = din("wd2Td", [2, 128, V])
    bd2d = din("bd2d", [128, 4])
    idxd = din("idxd", [BS, T], i32)
    egd = din("egd", [3, BS, T])
    epzd = din("epzd", [L, 4 * BS])
    lend = din("lend", [BS, 1])
    iotad = din("iotad", [BS, T])
    id32d = din("id32d", [BS, BS])
    id32rd = din("id32rd", [BS, BS], f32r)
    id128d = din("id128d", [128, 128])

    ew_d = nc.dram_tensor("ew_d", [V, 4 * H], f32r)

    enc_o = dout("enc_o", [S, BS, T, H])
    rec_o = dout("rec_o", [S, BS, T, V])
    mask_o = dout("mask_o", [S - 1, BT])
    blog_o = dout("blog_o", [S - 1, BT])
    bsamp_o = dout("bsamp_o", [S, BT])
    zlog_o = dout("zlog_o", [S, 2 * L, BS])
    zsamp_o = dout("zsamp_o", [S, L, BS])

    def r(ap):
        return ap.bitcast(f32r)

    with tile.TileContext(nc) as tc:
      with tc.tile_pool(name="persist", bufs=1) as pp:
        def ptile(tag, shape, dtype=f32):
            return pp.tile(shape, dtype, tag=tag, name=tag)

        whh0 = ptile("whh0", [128, 4 * H], f32r); whh1 = ptile("whh1", [128, 4 * H], f32r)
        encT0 = ptile("encT0", [128, BT], f32r); encT1 = ptile("encT1", [128, BT], f32r)
        SG = ptile("SG", [BS, 1024])   # sig_f | sig_i | tanh_g | sig_o
        CW = ptile("CW", [BS, 512])     # cu 0:256 | tanh(cu) 256:512
        UV = ptile("UV", [BS, 512])
        Mrow = ptile("Mrow", [BS, T])
        idx_s = ptile("idx_s", [BS, T], i32)
        id32 = ptile("id32", [BS, BS])
        id32r = ptile("id32r", [BS, BS], f32r)
        id128 = ptile("id128", [128, 128])
        eps_s = ptile("eps_s", [L, 4 * BS])
        wb1_0 = ptile("wb1_0", [128, H], f32r); wb1_1 = ptile("wb1_1", [128, H], f32r)
        wb2_s = ptile("wb2_s", [128, 2], f32r); bb1_s = ptile("bb1_s", [128, 2])
        bb2_s = ptile("bb2_s", [1, 1]); bb2c_s = ptile("bb2c_s", [1, 1])
        wz1_0 = ptile("wz1_0", [128, H]); wz1_1 = ptile("wz1_1", [128, H])
        bz1_s = ptile("bz1_s", [128, 2])
        wz2_0 = ptile("wz2_0", [128, 2 * L]); wz2_1 = ptile("wz2_1", [128, 2 * L])
        bz2_s = ptile("bz2_s", [128, 1])
        wd1_s = ptile("wd1_s", [L, H]); bd1_s = ptile("bd1_s", [128, 2])
        wd2_0 = ptile("wd2_0", [128, V]); wd2_1 = ptile("wd2_1", [128, V])
        bd2_s = ptile("bd2_s", [128, 4])
        LCA32 = ptile("LCA32", [BS, T]); LG32 = ptile("LG32", [BS, T])
        EG32 = ptile("EG32", [BS, T]); ones32 = ptile("ones32", [BS, T])
        iota32 = ptile("iota32", [BS, T]); len32c = ptile("len32c", [BS, 1])
        ones1 = ptile("ones1", [1, 128])
        epsb = ptile("epsb", [128, 1])
        EX32 = ptile("EX32", [BS, T]); SB32 = ptile("SB32", [BS, T])
        ssum = ptile("ssum", [BS, 1]); rsum = ptile("rsum", [BS, 1])
        PRD = ptile("PRD", [128, 512])
        rdT = ptile("rdT", [128, 2 * BS])
        ZT = ptile("ZT", [128, BS])
        ZW = ptile("ZW", [L, 3 * BS])
        szT_s = ptile("szT_s", [L, BS])
        z1_s = ptile("z1_s", [128, 2 * BS])
        d1_s = ptile("d1_s", [128, 2 * BS])
        pred_s = ptile("pred_s", [128, 4 * BS])
        prow = ptile("prow", [BS, V])

        sdma = nc.sync.dma_start
        for dst, src in [
            (whh0[:], whhTd[0]), (whh1[:], whhTd[1]),
            (idx_s[:], idxd[:, :]), (id32[:], id32d[:, :]), (id32r[:], id32rd[:, :]),
            (id128[:], id128d[:, :]), (eps_s[:], epzd[:, :]),
            (wb1_0[:], wb1Td[0]), (wb1_1[:], wb1Td[1]),
            (wb2_s[:], wb2d[:, :]), (bb1_s[:], bb1d[:, :]),
            (bb2_s[:], bb2d[:, :]),
            (wz1_0[:], wz1Td[0]), (wz1_1[:], wz1Td[1]), (bz1_s[:], bz1d[:, :]),
            (wz2_0[:], wz2Td[0]), (wz2_1[:], wz2Td[1]), (bz2_s[:], bz2d[:, :]),
            (wd1_s[:], wd1Td[:, :]), (bd1_s[:], bd1d[:, :]),
            (wd2_0[:], wd2Td[0]), (wd2_1[:], wd2Td[1]), (bd2_s[:], bd2d[:, :]),
            (iota32[:], iotad[:, :]), (len32c[:], lend[:, :]),
        ]:
            sdma(dst, src)
        nc.vector.memset(ones1[:], 1.0)
        nc.vector.memset(epsb[:], EPS)
        nc.vector.memset(ones32[:], 1.0)
        nc.vector.tensor_scalar_add(bb2c_s[:], bb2_s[:], -CSOFT)

        # ---------------- P0: EW table ----------------
        ew_dmas = []
        with (tc.tile_pool(name="p0sb", bufs=2) as p0sb,
              tc.tile_pool(name="p0ps", bufs=2, space="PSUM") as p0ps,
              tc.tile_pool(name="p0ps2", bufs=2, space="PSUM") as p0ps2):
            embT = [p0sb.tile([128, V], f32r, tag="embT0", name="embT0"),
                    p0sb.tile([128, V], f32r, tag="embT1", name="embT1")]
            wihs = [p0sb.tile([128, 4 * H], f32r, tag="wihs0", name="wihs0"),
                    p0sb.tile([128, 4 * H], f32r, tag="wihs1", name="wihs1")]
            sdma(wihs[0][:], wihT[0]); sdma(wihs[1][:], wihT[1])
            bsum = p0sb.tile([1, 4 * H], f32, tag="bsum", name="bsum")
            bih_t = p0sb.tile([1, 4 * H], f32, tag="bih_t", name="bih_t")
            bhh_t = p0sb.tile([1, 4 * H], f32, tag="bhh_t", name="bhh_t")
            sdma(bih_t[:], bihs[:, :]); sdma(bhh_t[:], bhhs[:, :])
            nc.vector.tensor_tensor(bsum[:], bih_t[:], bhh_t[:], op=OP.add)

            for vc in range(4):
                et = p0sb.tile([128, H], f32, tag="et", name="et")
                sdma(et[:], embw[128 * vc:128 * (vc + 1), :])
                for hc in range(2):
                    pt0 = p0ps.tile([128, 128], f32, tag="p0t", name="p0t")
                    nc.tensor.transpose(pt0[:], et[:, 128 * hc:128 * (hc + 1)],
                                        id128[:])
                    nc.scalar.copy(embT[hc][:, 128 * vc:128 * (vc + 1)], pt0[:])

            for vc in range(4):
                ewst = p0sb.tile([128, 4 * H], f32r, tag="ewst", name="ewst")
                for nb in range(2):
                    pe = p0ps2.tile([128, 512], f32, tag="p0e", name="p0e")
                    for k in range(2):
                        nc.tensor.matmul(
                            pe[:], lhsT=embT[k][:, 128 * vc:128 * (vc + 1)],
                            rhs=wihs[k][:, 512 * nb:512 * (nb + 1)],
                            start=(k == 0), stop=False)
                    nc.tensor.matmul(pe[:], lhsT=ones1[:, 0:128],
                                     rhs=bsum[:, 512 * nb:512 * (nb + 1)],
                                     start=False, stop=True)
                    nc.scalar.copy(ewst[:, 512 * nb:512 * (nb + 1)], pe[:])
                ew_dmas.append(sdma(ew_d[128 * vc:128 * (vc + 1), :], ewst[:]))

        # ---------------- segments ----------------
        with (tc.tile_pool(name="gx", bufs=6) as gxp,
              tc.tile_pool(name="h2", bufs=3) as h2p,
              tc.tile_pool(name="rows", bufs=4) as rows,
              tc.tile_pool(name="hidp", bufs=3) as hidp):

            for s in range(S):
                if s == 0:
                    nc.vector.memset(Mrow[:], 1.0)
                    nc.vector.memset(LCA32[:], 0.0)
                nc.vector.memset(CW[:, 0:256], 0.0)

                with (tc.tile_pool(name=f"pg{s}", bufs=2, space="PSUM") as pgp,
                      tc.tile_pool(name=f"pt{s}", bufs=2, space="PSUM") as ptp,
                      tc.tile_pool(name=f"pdum{s}", bufs=1, space="PSUM") as pdp):
                    pdum = pdp.tile([BS, 512], f32, tag="pdum", name="pdum")
                    # HAM warm-bridge: dense PE burst spanning the table-load
                    # stall at segment start so K=8/8 carries into the loop
                    for _ in range(10):
                        nc.tensor.matmul(pdum[:], lhsT=id32r[:],
                                         rhs=whh0[0:32, 0:512],
                                         start=True, stop=True)
                    for t in range(T):
                        gx = gxp.tile([BS, 4 * H], f32r, tag="gx", name="gx")
                        gi = nc.gpsimd.indirect_dma_start(
                            out=gx[:], out_offset=None,
                            in_=ew_d[:, :],
                            in_offset=IndirectOffsetOnAxis(
                                ap=idx_s[:, t:t + 1], axis=0))
                        if s == 0 and t == 0:
                            for d in ew_dmas:
                                tile.add_dep_helper(gi.ins, d.ins,
                                                    reason="gather after EW")
                        pg = pgp.tile([BS, 4 * H], f32, tag="pg", name="pg")
                        for nb in range(2):
                            pgn = pg[:, 512 * nb:512 * (nb + 1)]
                            if t > 0:
                                nc.tensor.matmul(
                                    pgn, lhsT=encT0[:, t - 1:BT:T],
                                    rhs=whh0[:, 512 * nb:512 * (nb + 1)],
                                    start=True, stop=False)
                                nc.tensor.matmul(
                                    pgn, lhsT=encT1[:, t - 1:BT:T],
                                    rhs=whh1[:, 512 * nb:512 * (nb + 1)],
                                    start=False, stop=False)
                                nc.tensor.matmul(
                                    pgn, lhsT=id32r[:],
                                    rhs=gx[:, 512 * nb:512 * (nb + 1)],
                                    start=False, stop=True)
                            else:
                                nc.tensor.matmul(
                                    pgn, lhsT=id32r[:],
                                    rhs=gx[:, 512 * nb:512 * (nb + 1)],
                                    start=True, stop=True)
                        # cols: sig_f 0:256 | sig_i 256:512 | tanh_g 512:768 | sig_o 768:1024
                        nc.scalar.activation(SG[:, 0:512], pg[:, 0:512],
                                             AF.Sigmoid)
                        nc.scalar.activation(SG[:, 512:768], pg[:, 512:768],
                                             AF.Tanh)
                        nc.scalar.activation(SG[:, 768:1024], pg[:, 768:1024],
                                             AF.Sigmoid)
                        nc.vector.scalar_tensor_tensor(
                            UV[:, 256:512], in0=SG[:, 0:256],
                            scalar=Mrow[:, t - 1:t] if t > 0 else 1.0,
                            in1=CW[:, 0:256], op0=OP.mult, op1=OP.mult)
                        nc.vector.tensor_tensor(UV[:, 0:256], SG[:, 256:512],
                                                SG[:, 512:768], op=OP.mult)
                        nc.tensor.matmul(pdum[:, 0:64], lhsT=id32[:],
                                         rhs=UV[:, 0:64], start=True, stop=True)
                        nc.vector.tensor_tensor(CW[:, 0:256], UV[:, 0:256],
                                                UV[:, 256:512], op=OP.add)
                        nc.scalar.activation(CW[:, 256:512], CW[:, 0:256],
                                             AF.Tanh)
                        nc.tensor.matmul(pdum[:, 0:64], lhsT=id32[:],
                                         rhs=UV[:, 256:320], start=True,
                                         stop=True)
                        h2 = h2p.tile([BS, H], f32, tag="h2", name="h2")
                        nc.vector.scalar_tensor_tensor(
                            h2[:], in0=CW[:, 256:512],
                            scalar=Mrow[:, t:t + 1],
                            in1=SG[:, 768:1024], op0=OP.mult, op1=OP.mult)
                        sdma(enc_o[s, :, t, :], h2[:])
                        ptt = ptp.tile([128, 2 * BS], f32, tag="ptt", name="ptt")
                        nc.tensor.transpose(ptt[:, 0:BS], h2[:, 0:128], id32[:])
                        nc.tensor.transpose(ptt[:, BS:2 * BS], h2[:, 128:256],
                                            id32[:])
                        nc.scalar.copy(encT0[:, t:BT:T], ptt[:, 0:BS])
                        nc.vector.tensor_copy(encT1[:, t:BT:T], ptt[:, BS:2 * BS])

                # ---------------- boundary ----------------
                if s < S - 1:
                    with (tc.tile_pool(name=f"ph{s}", bufs=4,
                                       space="PSUM") as php,
                          tc.tile_pool(name=f"pl{s}", bufs=2,
                                       space="PSUM") as plp,
                          tc.tile_pool(name=f"lbsl{s}", bufs=4) as lbp):
                        for nsl in range(8):
                            cs = slice(512 * nsl, 512 * (nsl + 1))
                            hid = [None, None]
                            for mch in range(2):
                                ph = php.tile([128, 512], f32, tag="ph",
                                              name="ph")
                                for k in range(2):
                                    enc_k = (encT0, encT1)[k]
                                    wb1_k = (wb1_0, wb1_1)[k]
                                    nc.tensor.matmul(
                                        ph[:],
                                        lhsT=wb1_k[:, 128 * mch:128 * (mch + 1)],
                                        rhs=enc_k[:, cs],
                                        start=(k == 0), stop=(k == 1))
                                hid[mch] = hidp.tile([128, 512], f32r,
                                                     tag="hid", name="hid")
                                nc.scalar.activation(
                                    hid[mch][:], ph[:], AF.Relu,
                                    bias=bb1_s[:, mch:mch + 1])
                            pl = plp.tile([1, 512], f32, tag="pl", name="pl")
                            for mch in range(2):
                                nc.tensor.matmul(
                                    pl[:], lhsT=wb2_s[:, mch:mch + 1],
                                    rhs=hid[mch][:],
                                    start=(mch == 0), stop=(mch == 1))
                            lbsl = lbp.tile([1, 512], f32, tag="lbsl",
                                            name="lbsl")
                            nc.scalar.activation(lbsl[:], pl[:],
                                                 AF.Identity, bias=bb2_s[:])
                            sdma(blog_o[s:s + 1, cs], lbsl[:])
                            exsl = lbp.tile([1, 512], f32, tag="exsl",
                                            name="exsl")
                            nc.scalar.activation(exsl[:], pl[:], AF.Exp,
                                                 bias=bb2c_s[:])
                            sdma(EX32[4 * nsl:4 * (nsl + 1), :], exsl[:])
                    sdma(EG32[:], egd[s])
                    nc.vector.memset(EX32[:, 0:1], 0.0)
                    nc.vector.tensor_tensor(EX32[:], EX32[:], EG32[:],
                                            op=OP.mult)
                    nc.vector.tensor_reduce(ssum[:], EX32[:], axis=AX.X,
                                            op=OP.add)
                    nc.vector.reciprocal(rsum[:], ssum[:])
                    nc.vector.tensor_scalar_mul(SB32[:], EX32[:], rsum[:])
                else:
                    nc.vector.tensor_scalar(SB32[:], iota32[:], len32c[:],
                                            None, op0=OP.is_equal)
                sdma(bsamp_o[s:s + 1, :], SB32[:])

                if s < S - 1:
                    # cumsum over t per batch row, then mask = exp(sum log)
                    nc.vector.tensor_tensor_scan(
                        EX32[:], data0=ones32[:], data1=SB32[:], initial=0.0,
                        op0=OP.mult, op1=OP.add)
                    nc.scalar.activation(LG32[:], EX32[:], AF.Ln, bias=epsb[0:BS, :])
                    nc.vector.tensor_tensor(LCA32[:], LCA32[:], LG32[:],
                                            op=OP.add)
                    nc.scalar.activation(Mrow[:], LCA32[:], AF.Exp)
                    sdma(mask_o[s:s + 1, :], Mrow[:])

                # ---- readout ----
                sbsh = rows.tile([1, BT], f32, tag="row", name="row")
                nc.vector.memset(sbsh[:], 0.0)
                sdma(sbsh[0:1, :].rearrange("p (b t) -> p b t", t=T)[:, :, 0:T - 1],
                     SB32[:, 1:T])
                with tc.tile_pool(name=f"pb{s}", bufs=2, space="PSUM") as pbp:
                    for nsl in range(8):
                        cs = slice(512 * nsl, 512 * (nsl + 1))
                        pb = pbp.tile([128, 512], f32, tag="pb", name="pb")
                        nc.tensor.matmul(pb[:], lhsT=ones1[:, 0:128],
                                         rhs=sbsh[0:1, cs],
                                         start=True, stop=True)
                        for k in range(2):
                            enc_k = (encT0, encT1)[k]
                            nc.vector.tensor_tensor(PRD[:], enc_k[:, cs],
                                                    pb[:], op=OP.mult)
                            nc.vector.tensor_reduce(
                                rdT[:, k * BS + 4 * nsl:k * BS + 4 * nsl + 4],
                                PRD[:].rearrange("p (b t) -> p b t", t=T),
                                axis=AX.X, op=OP.add)

                # ---- z head + decode ----
                with tc.tile_pool(name=f"pz{s}", bufs=1, space="PSUM") as pzp:
                    pz = pzp.tile([128, 2 * BS], f32, tag="pz", name="pz")
                    for mch in range(2):
                        for k in range(2):
                            wz1_k = (wz1_0, wz1_1)[k]
                            nc.tensor.matmul(
                                pz[:, BS * mch:BS * (mch + 1)],
                                lhsT=wz1_k[:, 128 * mch:128 * (mch + 1)],
                                rhs=rdT[:, BS * k:BS * (k + 1)],
                                start=(k == 0), stop=(k == 1))
                    for mch in range(2):
                        nc.scalar.activation(z1_s[:, BS * mch:BS * (mch + 1)],
                                             pz[:, BS * mch:BS * (mch + 1)],
                                             AF.Relu, bias=bz1_s[:, mch:mch + 1])
                    plz = pzp.tile([128, BS], f32, tag="plz", name="plz")
                    for k in range(2):
                        wz2_k = (wz2_0, wz2_1)[k]
                        nc.tensor.matmul(plz[:], lhsT=wz2_k[:],
                                         rhs=z1_s[:, BS * k:BS * (k + 1)],
                                         start=(k == 0), stop=(k == 1))
                    nc.scalar.activation(ZT[:], plz[:], AF.Identity,
                                         bias=bz2_s[:])
                    sdma(zlog_o[s], ZT[:])
                    # ZT rows: [0:64]=log_var, [64:128]=mu
                    nc.scalar.activation(ZW[:, 0:BS], ZT[0:L, :], AF.Exp,
                                         scale=0.5)
                    nc.vector.tensor_copy(ZW[:, BS:2 * BS], ZT[L:2 * L, :])
                    nc.vector.tensor_tensor(ZW[:, 2 * BS:3 * BS], ZW[:, 0:BS],
                                            eps_s[:, BS * s:BS * (s + 1)],
                                            op=OP.mult)
                    nc.vector.tensor_tensor(szT_s[:], ZW[:, 2 * BS:3 * BS],
                                            ZW[:, BS:2 * BS], op=OP.add)
                    sdma(zsamp_o[s], szT_s[:])

                    pd = pzp.tile([128, 2 * BS], f32, tag="pd", name="pd")
                    for mch in range(2):
                        nc.tensor.matmul(
                            pd[:, BS * mch:BS * (mch + 1)],
                            lhsT=wd1_s[:, 128 * mch:128 * (mch + 1)],
                            rhs=szT_s[:], start=True, stop=True)
                    for mch in range(2):
                        nc.scalar.activation(d1_s[:, BS * mch:BS * (mch + 1)],
                                             pd[:, BS * mch:BS * (mch + 1)],
                                             AF.Relu, bias=bd1_s[:, mch:mch + 1])
                    pp4 = pzp.tile([128, 4 * BS], f32, tag="pp4", name="pp4")
                    for mch in range(4):
                        for k in range(2):
                            wd2_k = (wd2_0, wd2_1)[k]
                            nc.tensor.matmul(
                                pp4[:, BS * mch:BS * (mch + 1)],
                                lhsT=wd2_k[:, 128 * mch:128 * (mch + 1)],
                                rhs=d1_s[:, BS * k:BS * (k + 1)],
                                start=(k == 0), stop=(k == 1))
                    for mch in range(4):
                        nc.scalar.activation(pred_s[:, BS * mch:BS * (mch + 1)],
                                             pp4[:, BS * mch:BS * (mch + 1)],
                                             AF.Identity,
                                             bias=bd2_s[:, mch:mch + 1])
                    ppt = pzp.tile([BS, V], f32, tag="ppt", name="ppt")
                    for mch in range(4):
                        nc.tensor.transpose(ppt[:, 128 * mch:128 * (mch + 1)],
                                            pred_s[:, BS * mch:BS * (mch + 1)],
                                            id128[:])
                    nc.scalar.copy(prow[:], ppt[:])
                for tb in range(8):
                    src = prow[:].rearrange("b (o v) -> b o v", o=1)
                    src = src.broadcast_to([BS, 16, V])
                    sdma(rec_o[s, :, 16 * tb:16 * (tb + 1), :], src)

    return nc


# ----------------------------------------------------------------------------
# Host side
# ----------------------------------------------------------------------------

def _host_noise():
    import jax
    import jax.numpy as jnp
    cpu = jax.local_devices(backend="cpu")[0]
    with jax.default_device(cpu):
        nkey = jax.random.key(42)
        gum, epz = [], []
        for seg in range(S):
            u = jax.random.uniform(jax.random.fold_in(nkey, 2 * seg), (B, T),
                                   jnp.float32)
            gum.append(np.asarray(-jnp.log(EPS - jnp.log(u + EPS))))
            epz.append(np.asarray(jax.random.normal(
                jax.random.fold_in(nkey, 2 * seg + 1), (B, L), jnp.float32)))
    return np.stack(gum), np.stack(epz)


def _reorder(w):
    i, f, g, o = np.split(w, 4, axis=0)
    return np.concatenate([f, i, g, o], axis=0)


def make_in_maps(inputs):
    f32 = np.float32
    if "noise" not in _CACHE:
        _CACHE["noise"] = _host_noise()
    gum, epz = _CACHE["noise"]
    eg = np.exp(gum).astype(f32)

    w_ih = np.asarray(inputs["w_ih"], f32); w_hh = np.asarray(inputs["w_hh"], f32)
    b_ih = np.asarray(inputs["b_ih"], f32); b_hh = np.asarray(inputs["b_hh"], f32)
    wihT = np.ascontiguousarray(_reorder(w_ih).T)
    whhT = np.ascontiguousarray(_reorder(w_hh).T)

    wz2 = np.asarray(inputs["wz2"], f32); bz2 = np.asarray(inputs["bz2"], f32)
    wz2_r = np.concatenate([wz2[L:], wz2[:L]], axis=0)
    bz2_r = np.concatenate([bz2[L:], bz2[:L]])

    def kchunk(a):
        return np.ascontiguousarray(np.stack([a[0:128], a[128:256]]))

    com = dict(
        embw=np.asarray(inputs["embed_w"], f32),
        wihT=kchunk(wihT).astype(f32),
        bihs=_reorder(b_ih)[None].astype(f32),
        bhhs=_reorder(b_hh)[None].astype(f32),
        whhTd=kchunk(whhT).astype(f32),
        wb1Td=kchunk(np.ascontiguousarray(np.asarray(inputs["wb1"], f32).T)),
        wb2d=np.ascontiguousarray(
            np.asarray(inputs["wb2"], f32)[0].reshape(2, 128).T),
        bb1d=np.ascontiguousarray(
            np.asarray(inputs["bb1"], f32).reshape(2, 128).T),
        bb2d=np.asarray(inputs["bb2"], f32).reshape(1, 1),
        wz1Td=kchunk(np.ascontiguousarray(np.asarray(inputs["wz1"], f32).T)),
        bz1d=np.ascontiguousarray(
            np.asarray(inputs["bz1"], f32).reshape(2, 128).T),
        wz2Td=kchunk(np.ascontiguousarray(wz2_r.T)),
        bz2d=np.ascontiguousarray(bz2_r.reshape(128, 1)),
        wd1Td=np.ascontiguousarray(np.asarray(inputs["wd1"], f32).T),
        bd1d=np.ascontiguousarray(
            np.asarray(inputs["bd1"], f32).reshape(2, 128).T),
        wd2Td=kchunk(np.ascontiguousarray(np.asarray(inputs["wd2"], f32).T)),
        bd2d=np.ascontiguousarray(
            np.asarray(inputs["bd2"], f32).reshape(4, 128).T),
        iotad=np.ascontiguousarray(
            np.broadcast_to(np.arange(T, dtype=f32), (BS, T))),
        id32d=np.eye(BS, dtype=f32),
        id32rd=np.eye(BS, dtype=f32),
        id128d=np.eye(128, dtype=f32),
    )

    idx_full = np.asarray(inputs["inputs"]).astype(np.int32)
    len_full = np.asarray(inputs["lengths"]).astype(np.int64)

    in_maps = []
    for core in range(NCORES):
        sh = slice(core * BS, (core + 1) * BS)
        m = dict(com)
        m["idxd"] = np.ascontiguousarray(idx_full[sh])
        m["egd"] = np.ascontiguousarray(eg[:3, sh])
        m["epzd"] = np.ascontiguousarray(
            np.concatenate([epz[s_, sh].T for s_ in range(S)], axis=1))
        m["lend"] = np.ascontiguousarray(
            (len_full[sh] - 1).astype(f32).reshape(BS, 1))
        in_maps.append(m)
    return in_maps


def unshard(results):
    f32 = np.float32
    encs = np.concatenate([r["enc_o"] for r in results], axis=1)
    recs = np.concatenate([r["rec_o"] for r in results], axis=1)
    masks = np.concatenate(
        [r["mask_o"].reshape(S - 1, BS, T) for r in results], axis=1)
    blog = np.concatenate(
        [r["blog_o"].reshape(S - 1, BS, T) for r in results], axis=1)
    blog[:, :, 0] = NEG_INF
    bsamp = np.concatenate(
        [r["bsamp_o"].reshape(S, BS, T) for r in results], axis=1)
    zl = np.concatenate([r["zlog_o"] for r in results], axis=2)
    zlog = np.concatenate([zl[:, L:], zl[:, :L]], axis=1).transpose(0, 2, 1)
    zsamp = np.concatenate([r["zsamp_o"] for r in results],
                           axis=2).transpose(0, 2, 1)
    return (encs.astype(f32), recs.astype(f32), masks.astype(f32),
            blog.astype(f32), bsamp.astype(f32), zlog.astype(f32),
            zsamp.astype(f32))


def kernel(**inputs):
    if "nc" not in _CACHE:
        _CACHE["nc"] = build_bass()
        _split_sync_waits(_CACHE["nc"], 1)
    nc = _CACHE["nc"]
    in_maps = make_in_maps(inputs)
    from concourse.bass_utils import run_bass_kernel_spmd
    res = run_bass_kernel_spmd(nc, in_maps, list(range(NCORES)),
                               trace=bool(os.environ.get("KTRACE")))
    _CACHE["last"] = res
    return unshard(res.results)


# revision 15
# speedup vs baseline: 1.3239x; 1.2785x over previous
"""Trainium2 Bass kernel for nn_CompILE (CompILE forward), 8-core data parallel.

Sharding: batch B=256 split across 8 NeuronCores (32 each); weights replicated.
Per core:
  P0: transpose embed_w on-device; build gate-embedding table
      EW[v, :] = w_ih_r @ embed_w[v] + (b_ih + b_hh) (gate order i,f,o,g) in DRAM.
  LSTM (4 segments x 128 sequential steps): gates = h @ w_hh_r.T + EW[idx_t]
      (PE matmuls, fp32r moving operand; EW row fetched by indirect DMA and
      added via identity matmul); sigmoid/tanh on ACT; cell update on DVE;
      PE-transposed h kept in SBUF as encT [H, b*T+t] for the boundary heads.
  Boundary: hid = relu(wb1 @ encT + b); lb = wb2 . hid; gumbel softmax as
      exp(lb - C)*exp(gumbel) normalized per row (host ships exp(gumbel),
      threefry seed 42, matching the reference); cumsum via tensor_tensor_scan;
      mask = exp(accumulated log cumsum); readout/z/decode in transposed form.
Host side only reorders/shards/transposes parameter layouts, precomputes the
fixed-seed noise constants, and reassembles device outputs.
"""
import os
import sys
import numpy as np

if "/opt/trn_rl_repo" not in sys.path:
    sys.path.insert(0, "/opt/trn_rl_repo")

EPS = 1e-17
NEG_INF = -1e30
B, T, V, H, L, S = 256, 128, 512, 256, 64, 4
BS = 32
BT = BS * T  # per-core rows, b-major: col = b*T + t
CSOFT = 10.0
NCORES = 8

_CACHE = {}


def _split_sync_waits(nc, limit=1):
    """walrus codegen in this build rejects >1 sync wait per instruction;
    hoist excess waits onto same-engine no-ops inserted just before."""
    import bass_rust
    import concourse.mybir as mybir
    n = 0
    for fn in nc.m.functions:
        for bb in fn.blocks:
            out = []
            changed = False
            for inst in bb.instructions:
                si = getattr(inst, "sync_info", None)
                ow = list(si.on_wait) if (si is not None and si.on_wait) else []
                if len(ow) > limit:
                    keep = ow[-limit:]
                    extra = ow[:-limit]
                    for j in range(0, len(extra), limit):
                        nop = bass_rust.InstNoOp(
                            name=f"I-wsplit-{n}", ins=[], outs=[])
                        n += 1
                        nop.engine = inst.engine
                        nop.sync_info = mybir.SyncInfo(
                            on_wait=extra[j:j + limit], on_update=[])
                        out.append(nop)
                    si.on_wait = keep
                    changed = True
                out.append(inst)
            if changed:
                try:
                    bb.instructions = out
                except Exception:
                    bb.instructions.clear()
                    for x in out:
                        bb.instructions.append(x)
    return n


def build_bass():
    import concourse.bass as bass
    import concourse.mybir as mybir
    import concourse.tile as tile
    from concourse.bass import IndirectOffsetOnAxis

    f32 = mybir.dt.float32
    f32r = mybir.dt.float32r
    i32 = mybir.dt.int32
    AF = mybir.ActivationFunctionType
    OP = mybir.AluOpType
    AX = mybir.AxisListType

    nc = bass.Bass("TRN2", target_bir_lowering=False, debug=False,
                   num_devices=NCORES)

    def din(name, shape, dtype=f32):
        return nc.dram_tensor(name, shape, dtype, kind="ExternalInput")

    def dout(name, shape, dtype=f32):
        return nc.dram_tensor(name, shape, dtype, kind="ExternalOutput")

    embw = din("embw", [V, H])
    wihT = din("wihT", [2, 128, 4 * H], f32r)
    bihs = din("bihs", [1, 4 * H])
    bhhs = din("bhhs", [1, 4 * H])
    whhTd = din("whhTd", [2, 128, 4 * H], f32r)
    wb1Td = din("wb1Td", [2, 128, H], f32r)
    wb2d = din("wb2d", [128, 2], f32r)
    bb1d = din("bb1d", [128, 2])
    bb2d = din("bb2d", [1, 1])
    wz1Td = din("wz1Td", [2, 128, H])
    bz1d = din("bz1d", [128, 2])
    wz2Td = din("wz2Td", [2, 128, 2 * L])   # rows reordered to [log_var; mu]
    bz2d = din("bz2d", [128, 1])
    wd1Td = din("wd1Td", [L, H])
    bd1d = din("bd1d", [128, 2])
    wd2Td = din("wd2Td", [2, 128, V])
    bd2d = din("bd2d", [128, 4])
    idxd = din("idxd", [BS, T], i32)
    egd = din("egd", [3, BS, T])
    epzd = din("epzd", [L, 4 * BS])
    lend = din("lend", [BS, 1])
    iotad = din("iotad", [BS, T])
    id32d = din("id32d", [BS, BS])
    id32rd = din("id32rd", [BS, BS], f32r)
    id128d = din("id128d", [128, 128])

    ew_d = nc.dram_tensor("ew_d", [V, 4 * H], f32r)

    enc_o = dout("enc_o", [S, BS, T, H])
    rec_o = dout("rec_o", [S, BS, T, V])
    mask_o = dout("mask_o", [S - 1, BT])
    blog_o = dout("blog_o", [S - 1, BT])
    bsamp_o = dout("bsamp_o", [S, BT])
    zlog_o = dout("zlog_o", [S, 2 * L, BS])
    zsamp_o = dout("zsamp_o", [S, L, BS])

    def r(ap):
        return ap.bitcast(f32r)

    with tile.TileContext(nc) as tc:
      with tc.tile_pool(name="persist", bufs=1) as pp:
        def ptile(tag, shape, dtype=f32):
            return pp.tile(shape, dtype, tag=tag, name=tag)

        whh0 = ptile("whh0", [128, 4 * H], f32r); whh1 = ptile("whh1", [128, 4 * H], f32r)
        encT0 = ptile("encT0", [128, BT], f32r); encT1 = ptile("encT1", [128, BT], f32r)
        SG = ptile("SG", [BS, 1024])   # sig_f | sig_i | tanh_g | sig_o
        CW = ptile("CW", [BS, 512])     # cu 0:256 | tanh(cu) 256:512
        UV = ptile("UV", [BS, 512])
        Mrow = ptile("Mrow", [BS, T])
        idx_s = ptile("idx_s", [BS, T], i32)
        id32 = ptile("id32", [BS, BS])
        id32r = ptile("id32r", [BS, BS], f32r)
        id128 = ptile("id128", [128, 128])
        eps_s = ptile("eps_s", [L, 4 * BS])
        wb1_0 = ptile("wb1_0", [128, H], f32r); wb1_1 = ptile("wb1_1", [128, H], f32r)
        wb2_s = ptile("wb2_s", [128, 2], f32r); bb1_s = ptile("bb1_s", [128, 2])
        bb2_s = ptile("bb2_s", [1, 1]); bb2c_s = ptile("bb2c_s", [1, 1])
        wz1_0 = ptile("wz1_0", [128, H]); wz1_1 = ptile("wz1_1", [128, H])
        bz1_s = ptile("bz1_s", [128, 2])
        wz2_0 = ptile("wz2_0", [128, 2 * L]); wz2_1 = ptile("wz2_1", [128, 2 * L])
        bz2_s = ptile("bz2_s", [128, 1])
        wd1_s = ptile("wd1_s", [L, H]); bd1_s = ptile("bd1_s", [128, 2])
        wd2_0 = ptile("wd2_0", [128, V]); wd2_1 = ptile("wd2_1", [128, V])
        bd2_s = ptile("bd2_s", [128, 4])
        LCA32 = ptile("LCA32", [BS, T]); LG32 = ptile("LG32", [BS, T])
        EG32 = ptile("EG32", [BS, T]); ones32 = ptile("ones32", [BS, T])
        iota32 = ptile("iota32", [BS, T]); len32c = ptile("len32c", [BS, 1])
        ones1 = ptile("ones1", [1, 128])
        epsb = ptile("epsb", [128, 1])
        EX32 = ptile("EX32", [BS, T]); SB32 = ptile("SB32", [BS, T])
        ssum = ptile("ssum", [BS, 1]); rsum = ptile("rsum", [BS, 1])
        PRD = ptile("PRD", [128, 512])
        rdT = ptile("rdT", [128, 2 * BS])
        ZT = ptile("ZT", [128, BS])
        ZW = ptile("ZW", [L, 3 * BS])
        szT_s = ptile("szT_s", [L, BS])
        z1_s = ptile("z1_s", [128, 2 * BS])
        d1_s = ptile("d1_s", [128, 2 * BS])
        pred_s = ptile("pred_s", [128, 4 * BS])
        prow = ptile("prow", [BS, V])

        sdma = nc.sync.dma_start
        for dst, src in [
            (whh0[:], whhTd[0]), (whh1[:], whhTd[1]),
            (idx_s[:], idxd[:, :]), (id32[:], id32d[:, :]), (id32r[:], id32rd[:, :]),
            (id128[:], id128d[:, :]), (eps_s[:], epzd[:, :]),
            (wb1_0[:], wb1Td[0]), (wb1_1[:], wb1Td[1]),
            (wb2_s[:], wb2d[:, :]), (bb1_s[:], bb1d[:, :]),
            (bb2_s[:], bb2d[:, :]),
            (wz1_0[:], wz1Td[0]), (wz1_1[:], wz1Td[1]), (bz1_s[:], bz1d[:, :]),
            (wz2_0[:], wz2Td[0]), (wz2_1[:], wz2Td[1]), (bz2_s[:], bz2d[:, :]),
            (wd1_s[:], wd1Td[:, :]), (bd1_s[:], bd1d[:, :]),
            (wd2_0[:], wd2Td[0]), (wd2_1[:], wd2Td[1]), (bd2_s[:], bd2d[:, :]),
            (iota32[:], iotad[:, :]), (len32c[:], lend[:, :]),
        ]:
            sdma(dst, src)
        nc.vector.memset(ones1[:], 1.0)
        nc.vector.memset(epsb[:], EPS)
        nc.vector.memset(ones32[:], 1.0)
        nc.vector.tensor_scalar_add(bb2c_s[:], bb2_s[:], -CSOFT)

        # ---------------- P0: EW table ----------------
        ew_dmas = []
        with (tc.tile_pool(name="p0sb", bufs=2) as p0sb,
              tc.tile_pool(name="p0ps", bufs=2, space="PSUM") as p0ps,
              tc.tile_pool(name="p0ps2", bufs=2, space="PSUM") as p0ps2):
            embT = [p0sb.tile([128, V], f32r, tag="embT0", name="embT0"),
                    p0sb.tile([128, V], f32r, tag="embT1", name="embT1")]
            wihs = [p0sb.tile([128, 4 * H], f32r, tag="wihs0", name="wihs0"),
                    p0sb.tile([128, 4 * H], f32r, tag="wihs1", name="wihs1")]
            sdma(wihs[0][:], wihT[0]); sdma(wihs[1][:], wihT[1])
            bsum = p0sb.tile([1, 4 * H], f32, tag="bsum", name="bsum")
            bih_t = p0sb.tile([1, 4 * H], f32, tag="bih_t", name="bih_t")
            bhh_t = p0sb.tile([1, 4 * H], f32, tag="bhh_t", name="bhh_t")
            sdma(bih_t[:], bihs[:, :]); sdma(bhh_t[:], bhhs[:, :])
            nc.vector.tensor_tensor(bsum[:], bih_t[:], bhh_t[:], op=OP.add)

            for vc in range(4):
                et = p0sb.tile([128, H], f32, tag="et", name="et")
                sdma(et[:], embw[128 * vc:128 * (vc + 1), :])
                for hc in range(2):
                    pt0 = p0ps.tile([128, 128], f32, tag="p0t", name="p0t")
                    nc.tensor.transpose(pt0[:], et[:, 128 * hc:128 * (hc + 1)],
                                        id128[:])
                    nc.scalar.copy(embT[hc][:, 128 * vc:128 * (vc + 1)], pt0[:])

            for vc in range(4):
                ewst = p0sb.tile([128, 4 * H], f32r, tag="ewst", name="ewst")
                for nb in range(2):
                    pe = p0ps2.tile([128, 512], f32, tag="p0e", name="p0e")
                    for k in range(2):
                        nc.tensor.matmul(
                            pe[:], lhsT=embT[k][:, 128 * vc:128 * (vc + 1)],
                            rhs=wihs[k][:, 512 * nb:512 * (nb + 1)],
                            start=(k == 0), stop=False)
                    nc.tensor.matmul(pe[:], lhsT=ones1[:, 0:128],
                                     rhs=bsum[:, 512 * nb:512 * (nb + 1)],
                                     start=False, stop=True)
                    nc.scalar.copy(ewst[:, 512 * nb:512 * (nb + 1)], pe[:])
                ew_dmas.append(sdma(ew_d[128 * vc:128 * (vc + 1), :], ewst[:]))

        # ---------------- segments ----------------
        with (tc.tile_pool(name="gx", bufs=6) as gxp,
              tc.tile_pool(name="h2", bufs=3) as h2p,
              tc.tile_pool(name="rows", bufs=4) as rows,
              tc.tile_pool(name="hidp", bufs=3) as hidp):

            for s in range(S):
                if s == 0:
                    nc.vector.memset(Mrow[:], 1.0)
                    nc.vector.memset(LCA32[:], 0.0)
                nc.vector.memset(CW[:, 0:256], 0.0)

                with (tc.tile_pool(name=f"pg{s}", bufs=2, space="PSUM") as pgp,
                      tc.tile_pool(name=f"pt{s}", bufs=2, space="PSUM") as ptp):
                    for t in range(T):
                        gx = gxp.tile([BS, 4 * H], f32r, tag="gx", name="gx")
                        gi = nc.gpsimd.indirect_dma_start(
                            out=gx[:], out_offset=None,
                            in_=ew_d[:, :],
                            in_offset=IndirectOffsetOnAxis(
                                ap=idx_s[:, t:t + 1], axis=0))
                        if s == 0 and t == 0:
                            for d in ew_dmas:
                                tile.add_dep_helper(gi.ins, d.ins,
                                                    reason="gather after EW")
                        pgF = pgp.tile([BS, 512], f32, tag="pgF", name="pgF")
                        pgG = pgp.tile([BS, 512], f32, tag="pgG", name="pgG")
                        for nb, pgn in ((0, pgF[:]), (1, pgG[:])):
                            nc.tensor.matmul(
                                pgn, lhsT=id32r[:],
                                rhs=gx[:, 512 * nb:512 * (nb + 1)],
                                start=True, stop=(t == 0))
                            if t > 0:
                                nc.tensor.matmul(
                                    pgn, lhsT=encT0[:, t - 1:BT:T],
                                    rhs=whh0[:, 512 * nb:512 * (nb + 1)],
                                    start=False, stop=False)
                                nc.tensor.matmul(
                                    pgn, lhsT=encT1[:, t - 1:BT:T],
                                    rhs=whh1[:, 512 * nb:512 * (nb + 1)],
                                    start=False, stop=True)
                        # pgF: sig_f 0:256 | sig_i 256:512 ; pgG: tanh_g 0:256 | sig_o 256:512
                        nc.scalar.activation(SG[:, 0:512], pgF[:], AF.Sigmoid)
                        nc.scalar.activation(SG[:, 512:768], pgG[:, 0:256],
                                             AF.Tanh)
                        nc.scalar.activation(SG[:, 768:1024], pgG[:, 256:512],
                                             AF.Sigmoid)
                        nc.vector.scalar_tensor_tensor(
                            UV[:, 256:512], in0=SG[:, 0:256],
                            scalar=Mrow[:, t - 1:t] if t > 0 else 1.0,
                            in1=CW[:, 0:256], op0=OP.mult, op1=OP.mult)
                        nc.vector.tensor_tensor(UV[:, 0:256], SG[:, 256:512],
                                                SG[:, 512:768], op=OP.mult)
                        nc.vector.tensor_tensor(CW[:, 0:256], UV[:, 0:256],
                                                UV[:, 256:512], op=OP.add)
                        h2 = h2p.tile([BS, H], f32, tag="h2", name="h2")
                        ptt = ptp.tile([128, 2 * BS], f32, tag="ptt", name="ptt")
                        for hh in range(2):
                            hs = slice(128 * hh, 128 * (hh + 1))
                            nc.scalar.activation(CW[:, 256 + 128 * hh:384 + 128 * hh],
                                                 CW[:, 128 * hh:128 * (hh + 1)],
                                                 AF.Tanh)
                            nc.vector.scalar_tensor_tensor(
                                h2[:, hs], in0=CW[:, 256 + 128 * hh:384 + 128 * hh],
                                scalar=Mrow[:, t:t + 1],
                                in1=SG[:, 768 + 128 * hh:896 + 128 * hh],
                                op0=OP.mult, op1=OP.mult)
                            nc.tensor.transpose(ptt[:, BS * hh:BS * (hh + 1)],
                                                h2[:, hs], id32[:])
                        nc.scalar.copy(encT0[:, t:BT:T], ptt[:, 0:BS])
                        nc.vector.tensor_copy(encT1[:, t:BT:T], ptt[:, BS:2 * BS])
                        sdma(enc_o[s, :, t, :], h2[:])

                # ---------------- boundary ----------------
                if s < S - 1:
                    with (tc.tile_pool(name=f"ph{s}", bufs=4,
                                       space="PSUM") as php,
                          tc.tile_pool(name=f"pl{s}", bufs=2,
                                       space="PSUM") as plp,
                          tc.tile_pool(name=f"lbsl{s}", bufs=4) as lbp):
                        for nsl in range(8):
                            cs = slice(512 * nsl, 512 * (nsl + 1))
                            hid = [None, None]
                            for mch in range(2):
                                ph = php.tile([128, 512], f32, tag="ph",
                                              name="ph")
                                for k in range(2):
                                    enc_k = (encT0, encT1)[k]
                                    wb1_k = (wb1_0, wb1_1)[k]
                                    nc.tensor.matmul(
                                        ph[:],
                                        lhsT=wb1_k[:, 128 * mch:128 * (mch + 1)],
                                        rhs=enc_k[:, cs],
                                        start=(k == 0), stop=(k == 1))
                                hid[mch] = hidp.tile([128, 512], f32r,
                                                     tag="hid", name="hid")
                                nc.scalar.activation(
                                    hid[mch][:], ph[:], AF.Relu,
                                    bias=bb1_s[:, mch:mch + 1])
                            pl = plp.tile([1, 512], f32, tag="pl", name="pl")
                            for mch in range(2):
                                nc.tensor.matmul(
                                    pl[:], lhsT=wb2_s[:, mch:mch + 1],
                                    rhs=hid[mch][:],
                                    start=(mch == 0), stop=(mch == 1))
                            lbsl = lbp.tile([1, 512], f32, tag="lbsl",
                                            name="lbsl")
                            nc.scalar.activation(lbsl[:], pl[:],
                                                 AF.Identity, bias=bb2_s[:])
                            sdma(blog_o[s:s + 1, cs], lbsl[:])
                            exsl = lbp.tile([1, 512], f32, tag="exsl",
                                            name="exsl")
                            nc.scalar.activation(exsl[:], pl[:], AF.Exp,
                                                 bias=bb2c_s[:])
                            sdma(EX32[4 * nsl:4 * (nsl + 1), :], exsl[:])
                    sdma(EG32[:], egd[s])
                    nc.vector.memset(EX32[:, 0:1], 0.0)
                    nc.vector.tensor_tensor(EX32[:], EX32[:], EG32[:],
                                            op=OP.mult)
                    nc.vector.tensor_reduce(ssum[:], EX32[:], axis=AX.X,
                                            op=OP.add)
                    nc.vector.reciprocal(rsum[:], ssum[:])
                    nc.vector.tensor_scalar_mul(SB32[:], EX32[:], rsum[:])
                else:
                    nc.vector.tensor_scalar(SB32[:], iota32[:], len32c[:],
                                            None, op0=OP.is_equal)
                sdma(bsamp_o[s:s + 1, :], SB32[:])

                if s < S - 1:
                    # cumsum over t per batch row, then mask = exp(sum log)
                    nc.vector.tensor_tensor_scan(
                        EX32[:], data0=ones32[:], data1=SB32[:], initial=0.0,
                        op0=OP.mult, op1=OP.add)
                    nc.scalar.activation(LG32[:], EX32[:], AF.Ln, bias=epsb[0:BS, :])
                    nc.vector.tensor_tensor(LCA32[:], LCA32[:], LG32[:],
                                            op=OP.add)
                    nc.scalar.activation(Mrow[:], LCA32[:], AF.Exp)
                    sdma(mask_o[s:s + 1, :], Mrow[:])

                # ---- readout ----
                sbsh = rows.tile([1, BT], f32, tag="row", name="row")
                nc.vector.memset(sbsh[:], 0.0)
                sdma(sbsh[0:1, :].rearrange("p (b t) -> p b t", t=T)[:, :, 0:T - 1],
                     SB32[:, 1:T])
                with tc.tile_pool(name=f"pb{s}", bufs=2, space="PSUM") as pbp:
                    for nsl in range(8):
                        cs = slice(512 * nsl, 512 * (nsl + 1))
                        pb = pbp.tile([128, 512], f32, tag="pb", name="pb")
                        nc.tensor.matmul(pb[:], lhsT=ones1[:, 0:128],
                                         rhs=sbsh[0:1, cs],
                                         start=True, stop=True)
                        for k in range(2):
                            enc_k = (encT0, encT1)[k]
                            nc.vector.tensor_tensor(PRD[:], enc_k[:, cs],
                                                    pb[:], op=OP.mult)
                            nc.vector.tensor_reduce(
                                rdT[:, k * BS + 4 * nsl:k * BS + 4 * nsl + 4],
                                PRD[:].rearrange("p (b t) -> p b t", t=T),
                                axis=AX.X, op=OP.add)

                # ---- z head + decode ----
                with tc.tile_pool(name=f"pz{s}", bufs=1, space="PSUM") as pzp:
                    pz = pzp.tile([128, 2 * BS], f32, tag="pz", name="pz")
                    for mch in range(2):
                        for k in range(2):
                            wz1_k = (wz1_0, wz1_1)[k]
                            nc.tensor.matmul(
                                pz[:, BS * mch:BS * (mch + 1)],
                                lhsT=wz1_k[:, 128 * mch:128 * (mch + 1)],
                                rhs=rdT[:, BS * k:BS * (k + 1)],
                                start=(k == 0), stop=(k == 1))
                    for mch in range(2):
                        nc.scalar.activation(z1_s[:, BS * mch:BS * (mch + 1)],
                                             pz[:, BS * mch:BS * (mch + 1)],
                                             AF.Relu, bias=bz1_s[:, mch:mch + 1])
                    plz = pzp.tile([128, BS], f32, tag="plz", name="plz")
                    for k in range(2):
                        wz2_k = (wz2_0, wz2_1)[k]
                        nc.tensor.matmul(plz[:], lhsT=wz2_k[:],
                                         rhs=z1_s[:, BS * k:BS * (k + 1)],
                                         start=(k == 0), stop=(k == 1))
                    nc.scalar.activation(ZT[:], plz[:], AF.Identity,
                                         bias=bz2_s[:])
                    sdma(zlog_o[s], ZT[:])
                    # ZT rows: [0:64]=log_var, [64:128]=mu
                    nc.scalar.activation(ZW[:, 0:BS], ZT[0:L, :], AF.Exp,
                                         scale=0.5)
                    nc.vector.tensor_copy(ZW[:, BS:2 * BS], ZT[L:2 * L, :])
                    nc.vector.tensor_tensor(ZW[:, 2 * BS:3 * BS], ZW[:, 0:BS],
                                            eps_s[:, BS * s:BS * (s + 1)],
                                            op=OP.mult)
                    nc.vector.tensor_tensor(szT_s[:], ZW[:, 2 * BS:3 * BS],
                                            ZW[:, BS:2 * BS], op=OP.add)
                    sdma(zsamp_o[s], szT_s[:])

                    pd = pzp.tile([128, 2 * BS], f32, tag="pd", name="pd")
                    for mch in range(2):
                        nc.tensor.matmul(
                            pd[:, BS * mch:BS * (mch + 1)],
                            lhsT=wd1_s[:, 128 * mch:128 * (mch + 1)],
                            rhs=szT_s[:], start=True, stop=True)
                    for mch in range(2):
                        nc.scalar.activation(d1_s[:, BS * mch:BS * (mch + 1)],
                                             pd[:, BS * mch:BS * (mch + 1)],
                                             AF.Relu, bias=bd1_s[:, mch:mch + 1])
                    pp4 = pzp.tile([128, 4 * BS], f32, tag="pp4", name="pp4")
                    for mch in range(4):
                        for k in range(2):
                            wd2_k = (wd2_0, wd2_1)[k]
                            nc.tensor.matmul(
                                pp4[:, BS * mch:BS * (mch + 1)],
                                lhsT=wd2_k[:, 128 * mch:128 * (mch + 1)],
                                rhs=d1_s[:, BS * k:BS * (k + 1)],
                                start=(k == 0), stop=(k == 1))
                    for mch in range(4):
                        nc.scalar.activation(pred_s[:, BS * mch:BS * (mch + 1)],
                                             pp4[:, BS * mch:BS * (mch + 1)],
                                             AF.Identity,
                                             bias=bd2_s[:, mch:mch + 1])
                    ppt = pzp.tile([BS, V], f32, tag="ppt", name="ppt")
                    for mch in range(4):
                        nc.tensor.transpose(ppt[:, 128 * mch:128 * (mch + 1)],
                                            pred_s[:, BS * mch:BS * (mch + 1)],
                                            id128[:])
                    nc.scalar.copy(prow[:], ppt[:])
                for tb in range(8):
                    src = prow[:].rearrange("b (o v) -> b o v", o=1)
                    src = src.broadcast_to([BS, 16, V])
                    sdma(rec_o[s, :, 16 * tb:16 * (tb + 1), :], src)

    return nc


# ----------------------------------------------------------------------------
# Host side
# ----------------------------------------------------------------------------

def _host_noise():
    import jax
    import jax.numpy as jnp
    cpu = jax.local_devices(backend="cpu")[0]
    with jax.default_device(cpu):
        nkey = jax.random.key(42)
        gum, epz = [], []
        for seg in range(S):
            u = jax.random.uniform(jax.random.fold_in(nkey, 2 * seg), (B, T),
                                   jnp.float32)
            gum.append(np.asarray(-jnp.log(EPS - jnp.log(u + EPS))))
            epz.append(np.asarray(jax.random.normal(
                jax.random.fold_in(nkey, 2 * seg + 1), (B, L), jnp.float32)))
    return np.stack(gum), np.stack(epz)


def _reorder(w):
    i, f, g, o = np.split(w, 4, axis=0)
    return np.concatenate([f, i, g, o], axis=0)


def make_in_maps(inputs):
    f32 = np.float32
    if "noise" not in _CACHE:
        _CACHE["noise"] = _host_noise()
    gum, epz = _CACHE["noise"]
    eg = np.exp(gum).astype(f32)

    w_ih = np.asarray(inputs["w_ih"], f32); w_hh = np.asarray(inputs["w_hh"], f32)
    b_ih = np.asarray(inputs["b_ih"], f32); b_hh = np.asarray(inputs["b_hh"], f32)
    wihT = np.ascontiguousarray(_reorder(w_ih).T)
    whhT = np.ascontiguousarray(_reorder(w_hh).T)

    wz2 = np.asarray(inputs["wz2"], f32); bz2 = np.asarray(inputs["bz2"], f32)
    wz2_r = np.concatenate([wz2[L:], wz2[:L]], axis=0)
    bz2_r = np.concatenate([bz2[L:], bz2[:L]])

    def kchunk(a):
        return np.ascontiguousarray(np.stack([a[0:128], a[128:256]]))

    com = dict(
        embw=np.asarray(inputs["embed_w"], f32),
        wihT=kchunk(wihT).astype(f32),
        bihs=_reorder(b_ih)[None].astype(f32),
        bhhs=_reorder(b_hh)[None].astype(f32),
        whhTd=kchunk(whhT).astype(f32),
        wb1Td=kchunk(np.ascontiguousarray(np.asarray(inputs["wb1"], f32).T)),
        wb2d=np.ascontiguousarray(
            np.asarray(inputs["wb2"], f32)[0].reshape(2, 128).T),
        bb1d=np.ascontiguousarray(
            np.asarray(inputs["bb1"], f32).reshape(2, 128).T),
        bb2d=np.asarray(inputs["bb2"], f32).reshape(1, 1),
        wz1Td=kchunk(np.ascontiguousarray(np.asarray(inputs["wz1"], f32).T)),
        bz1d=np.ascontiguousarray(
            np.asarray(inputs["bz1"], f32).reshape(2, 128).T),
        wz2Td=kchunk(np.ascontiguousarray(wz2_r.T)),
        bz2d=np.ascontiguousarray(bz2_r.reshape(128, 1)),
        wd1Td=np.ascontiguousarray(np.asarray(inputs["wd1"], f32).T),
        bd1d=np.ascontiguousarray(
            np.asarray(inputs["bd1"], f32).reshape(2, 128).T),
        wd2Td=kchunk(np.ascontiguousarray(np.asarray(inputs["wd2"], f32).T)),
        bd2d=np.ascontiguousarray(
            np.asarray(inputs["bd2"], f32).reshape(4, 128).T),
        iotad=np.ascontiguousarray(
            np.broadcast_to(np.arange(T, dtype=f32), (BS, T))),
        id32d=np.eye(BS, dtype=f32),
        id32rd=np.eye(BS, dtype=f32),
        id128d=np.eye(128, dtype=f32),
    )

    idx_full = np.asarray(inputs["inputs"]).astype(np.int32)
    len_full = np.asarray(inputs["lengths"]).astype(np.int64)

    in_maps = []
    for core in range(NCORES):
        sh = slice(core * BS, (core + 1) * BS)
        m = dict(com)
        m["idxd"] = np.ascontiguousarray(idx_full[sh])
        m["egd"] = np.ascontiguousarray(eg[:3, sh])
        m["epzd"] = np.ascontiguousarray(
            np.concatenate([epz[s_, sh].T for s_ in range(S)], axis=1))
        m["lend"] = np.ascontiguousarray(
            (len_full[sh] - 1).astype(f32).reshape(BS, 1))
        in_maps.append(m)
    return in_maps


def unshard(results):
    f32 = np.float32
    encs = np.concatenate([r["enc_o"] for r in results], axis=1)
    recs = np.concatenate([r["rec_o"] for r in results], axis=1)
    masks = np.concatenate(
        [r["mask_o"].reshape(S - 1, BS, T) for r in results], axis=1)
    blog = np.concatenate(
        [r["blog_o"].reshape(S - 1, BS, T) for r in results], axis=1)
    blog[:, :, 0] = NEG_INF
    bsamp = np.concatenate(
        [r["bsamp_o"].reshape(S, BS, T) for r in results], axis=1)
    zl = np.concatenate([r["zlog_o"] for r in results], axis=2)
    zlog = np.concatenate([zl[:, L:], zl[:, :L]], axis=1).transpose(0, 2, 1)
    zsamp = np.concatenate([r["zsamp_o"] for r in results],
                           axis=2).transpose(0, 2, 1)
    return (encs.astype(f32), recs.astype(f32), masks.astype(f32),
            blog.astype(f32), bsamp.astype(f32), zlog.astype(f32),
            zsamp.astype(f32))


def kernel(**inputs):
    if "nc" not in _CACHE:
        _CACHE["nc"] = build_bass()
        _split_sync_waits(_CACHE["nc"], 1)
    nc = _CACHE["nc"]
    in_maps = make_in_maps(inputs)
    from concourse.bass_utils import run_bass_kernel_spmd
    res = run_bass_kernel_spmd(nc, in_maps, list(range(NCORES)),
                               trace=bool(os.environ.get("KTRACE")))
    _CACHE["last"] = res
    return unshard(res.results)
